# revision 37
# baseline (speedup 1.0000x reference)
"""NonLocalBlock (GroupNorm + 4096-token self-attention + proj + residual) on 8 TRN2 cores.

Sharding: core = (batch b in {0,1}, query-chunk q in {0..3}); each core holds its
batch's full x (needed for GN stats and K/V over all tokens) and computes the
output for its 1024-token query chunk. No collectives needed.

Math notes (exact reductions of the reference):
  - bk drops out: adding a per-j-constant... no -- adding k-bias shifts every
    logit of row i by q_i . bk, constant in j -> softmax invariant.
  - bv folds into the projection bias: softmax rows sum to 1, so
    proj(A + bv) = proj(A) + wp @ bv.
  - Normalization by the softmax row-sum commutes with the V- and P-matmuls,
    so we divide once on the small [c, i] result instead of the [i, j] matrix.
"""

import sys

for _p in ("/opt/trn_rl_repo",):
    if _p not in sys.path:
        sys.path.insert(0, _p)

import numpy as np

import concourse.bacc as bacc
import concourse.bass as bass
import concourse.tile as tile
from concourse import mybir
from concourse.bass_utils import run_bass_kernel_spmd

F32 = mybir.dt.float32
F32R = mybir.dt.float32r
AF = mybir.ActivationFunctionType
OP = mybir.AluOpType

B, C, T, H, W = 2, 256, 4, 32, 32
N = T * H * W            # 4096 tokens
NQ = N // 4              # 1024 query tokens per core
P = 128                  # partitions
CT = C // P              # 2 channel tiles
JT = N // P              # 32 key tiles of 128
NB = N // 512            # 8 key blocks of 512
IC = NQ // 512           # 2 query sub-chunks of 512
NGROUPS = 32
GSIZE = C // NGROUPS     # 8 channels per group
EPS = 1e-6
SCALE = C ** (-0.5)      # 1/16
# Pack the M=1 rowsum matmuls 4-at-a-time into disjoint PE column groups
# (tile_position) so they run concurrently -- each costs N cycles otherwise.
RS_PACK = False


def r(ap):
    """View an fp32 AP as float32r for full-rate PE matmuls (moving dim >= 256)."""
    return ap.bitcast(F32R)


def build_program(dbg=False):
    nc = bacc.Bacc("TRN2", target_bir_lowering=False, debug=False, num_devices=8)

    # ---- DRAM parameters (per core) ----
    xb_d = nc.declare_dram_parameter("xb", [CT, P, N], F32, isOutput=False)
    xq_d = nc.declare_dram_parameter("xq", [CT, P, NQ], F32, isOutput=False)
    wqT_d = nc.declare_dram_parameter("wqT", [CT, P, C], F32R, isOutput=False)
    wkT_d = nc.declare_dram_parameter("wkT", [CT, P, C], F32R, isOutput=False)
    wvT_d = nc.declare_dram_parameter("wvT", [CT, P, C], F32R, isOutput=False)
    wpT_d = nc.declare_dram_parameter("wpT", [CT, P, C], F32R, isOutput=False)
    # Packed small constants, one DMA: cols [0:32]=G group-indicator/GSIZE,
    # 32=bq, 33=bp, 34=gn_scale, 35=gn_bias.
    csm_d = nc.declare_dram_parameter("csm", [CT, P, NGROUPS + 5], F32,
                                      isOutput=False)
    bv_d = nc.declare_dram_parameter("bv", [CT, P, 2], F32R, isOutput=False)
    # GT[g, c] = (c//GSIZE == g)  (broadcast group stat back to channels)
    GT_d = nc.declare_dram_parameter("GT", [NGROUPS, C], F32, isOutput=False)
    out_d = nc.declare_dram_parameter("out", [CT, P, NQ], F32, isOutput=True)
    if dbg:
        dbg_h = nc.declare_dram_parameter("dbg_h", [CT, P, N], F32, isOutput=True)
        dbg_k = nc.declare_dram_parameter("dbg_k", [CT, P, N], F32, isOutput=True)
        dbg_vt = nc.declare_dram_parameter("dbg_vt", [JT, P, C], F32, isOutput=True)
        dbg_q = nc.declare_dram_parameter("dbg_q", [CT, P, NQ], F32, isOutput=True)
        dbg_s = nc.declare_dram_parameter("dbg_s", [P, 512], F32, isOutput=True)

    with tile.TileContext(nc) as tc:
        with (
            nc.allow_low_precision(reason="float32r rounding for full-rate PE"),
            tc.tile_pool(name="consts", bufs=1) as consts,
            tc.tile_pool(name="data", bufs=1) as data,
            tc.tile_pool(name="stats", bufs=1) as stats,
            tc.tile_pool(name="ptiles", bufs=8) as ptiles,
            tc.tile_pool(name="astiles", bufs=2) as astiles,
        ):
            # ---- input DMAs, one queue, explicit order by first-use time.
            # The ~330GB/s DMA pipe is the head bottleneck: small consts + wk
            # first (they gate the first PE ops), then the 4MB xb stream that
            # gates GN stats, then tensors needed progressively later.
            csm_sb = consts.tile([P, CT, NGROUPS + 5], F32, tag="csm")
            nc.sync.dma_start(out=csm_sb[:, :, :],
                              in_=csm_d.rearrange("ct p k -> p ct k"))
            G_sb = csm_sb[:, :, 0:NGROUPS]
            bq_sb = csm_sb[:, :, NGROUPS + 0]
            bp_sb = csm_sb[:, :, NGROUPS + 1]
            gsc_sb = csm_sb[:, :, NGROUPS + 2]
            gbi_sb = csm_sb[:, :, NGROUPS + 3]
            ngsc_sb = csm_sb[:, :, NGROUPS + 4]
            GT_sb = consts.tile([NGROUPS, C], F32, tag="GT")
            nc.sync.dma_start(out=GT_sb[:, :], in_=GT_d[:])
            bv_sb = consts.tile([P, CT, 2], F32R, tag="bv")
            nc.sync.dma_start(out=bv_sb[:, :, :],
                              in_=bv_d.rearrange("ct p k -> p ct k"))
            wq_sb = consts.tile([P, CT, C], F32R, tag="wq")
            wk_sb = consts.tile([P, CT, C], F32R, tag="wk")
            wv_sb = consts.tile([P, CT, C], F32R, tag="wv")
            wp_sb = consts.tile([P, CT, C], F32R, tag="wp")
            nc.sync.dma_start(out=wk_sb[:, :, :],
                              in_=wkT_d.rearrange("ct p o -> p ct o"))
            xb_sb = data.tile([P, CT, N], F32, tag="xb")      # raw x (stage 1 only)
            xq_sb = data.tile([P, CT, NQ], F32, tag="xq")
            for nb in range(NB):
                nsl = slice(nb * 512, (nb + 1) * 512)
                for ct in range(CT):
                    nc.sync.dma_start(out=xb_sb[:, ct, nsl], in_=xb_d[ct, :, nsl])
            nc.sync.dma_start(out=xq_sb[:, :, :],
                              in_=xq_d.rearrange("ct p i -> p ct i"))
            nc.sync.dma_start(out=wv_sb[:, :, :],
                              in_=wvT_d.rearrange("ct p o -> p ct o"))
            nc.sync.dma_start(out=wq_sb[:, :, :],
                              in_=wqT_d.rearrange("ct p o -> p ct o"))
            nc.sync.dma_start(out=wp_sb[:, :, :],
                              in_=wpT_d.rearrange("ct p o -> p ct o"))
            ones_f = consts.tile([P, 1], F32, tag="ones_f")
            nc.vector.memset(ones_f[:, :], 1.0)
            ones_sb = consts.tile([P, 1], F32, tag="ones")
            nc.vector.tensor_copy(ones_sb[:, :].bitcast(F32R), ones_f[:, :])
            epsg_sb = consts.tile([NGROUPS, 1], F32, tag="epsg")
            nc.vector.memset(epsg_sb[:, :], EPS)

            # ---- big SBUF tensors ----
            h_sb = data.tile([P, CT, N], F32, tag="h")        # GN output
            hq_sb = data.tile([P, CT, NQ], F32, tag="hq")
            k_sb = data.tile([P, CT, N], F32, tag="k")        # K[o, j]
            # vt reuses xb's slot (same tag/size): xb is dead once h is built
            vt_sb = data.tile([P, JT, C], F32, tag="xb")      # V^T[j, o]
            q_sb = data.tile([P, CT, NQ], F32, tag="q")       # Q[o, i]
            out_sb = data.tile([P, CT, NQ], F32, tag="out")

            # ================= Stage 1: GroupNorm =================
            with tc.tile_pool(name="ps1", bufs=1, space="PSUM") as ps1:
                # per-channel mean/var over the 4096 free positions
                bst = stats.tile([P, CT, NB, 6], F32, tag="bst")
                mv = stats.tile([P, CT, 2], F32, tag="mv")
                mst = stats.tile([P, CT, 2], F32, tag="mst")   # (mean_c, E[x^2]_c)
                # nb-major to match DMA chunk arrival order (DVE is in-order)
                for nb in range(NB):
                    for ct in range(CT):
                        nc.vector.bn_stats(
                            out=bst[:, ct, nb, :],
                            in_=xb_sb[:, ct, nb * 512:(nb + 1) * 512],
                        )
                for ct in range(CT):
                    nc.vector.bn_aggr(out=mv[:, ct, :], in_=bst[:, ct, :, :])
                    nc.vector.tensor_copy(mst[:, ct, 0:1], mv[:, ct, 0:1])
                    # E[x^2] = var + mean^2
                    nc.vector.tensor_tensor(
                        out=mst[:, ct, 1:2], in0=mv[:, ct, 0:1],
                        in1=mv[:, ct, 0:1], op=OP.mult)
                    nc.vector.tensor_tensor(
                        out=mst[:, ct, 1:2], in0=mst[:, ct, 1:2],
                        in1=mv[:, ct, 1:2], op=OP.add)
                # group-sum across partitions: [g, (mean, Ex2)]
                gps = ps1.tile([NGROUPS, 2], F32, tag="gps")
                for ct in range(CT):
                    nc.tensor.matmul(gps[:, :], G_sb[:, ct, :], mst[:, ct, :],
                                     start=(ct == 0), stop=(ct == CT - 1))
                gmv = stats.tile([NGROUPS, 2], F32, tag="gmv")
                nc.vector.tensor_copy(gmv[:, :], gps[:, :])
                gtmp = stats.tile([NGROUPS, 1], F32, tag="gtmp")
                gvec = stats.tile([NGROUPS, 2], F32, tag="gvec")  # (mean_g, rstd_g)
                nc.vector.tensor_copy(gvec[:, 0:1], gmv[:, 0:1])
                # -var = mean^2 - E[x^2]; sqrt(var+eps) via scale=-1
                nc.vector.scalar_tensor_tensor(
                    out=gtmp, in0=gmv[:, 0:1], scalar=gmv[:, 0:1],
                    in1=gmv[:, 1:2], op0=OP.mult, op1=OP.subtract)
                nc.scalar.activation(out=gtmp, in_=gtmp, func=AF.Sqrt,
                                     bias=epsg_sb[:, :], scale=-1.0)
                nc.vector.reciprocal(out=gvec[:, 1:2], in_=gtmp)  # rstd_g
                # broadcast back to channels: cb[c, (mean, rstd)]
                svec = stats.tile([P, CT], F32, tag="svec")
                nsvec = stats.tile([P, CT], F32, tag="nsvec")
                tvec = stats.tile([P, CT], F32, tag="tvec")
                for ct in range(CT):
                    cps = ps1.tile([P, 2], F32, tag="cps")
                    nc.tensor.matmul(cps[:, :], GT_sb[:, ct * P:(ct + 1) * P],
                                     gvec[:, :], start=True, stop=True)
                    cb = stats.tile([P, 2], F32, tag="cb")
                    nc.vector.tensor_copy(cb[:, :], cps[:, :])
                    # s_c = rstd_c * gn_scale_c ; t_c = gn_bias_c + mean_c*(-s_c)
                    nc.vector.tensor_tensor(out=svec[:, ct:ct + 1], in0=cb[:, 1:2],
                                            in1=gsc_sb[:, ct, None], op=OP.mult)
                    nc.vector.tensor_tensor(out=nsvec[:, ct:ct + 1], in0=cb[:, 1:2],
                                            in1=ngsc_sb[:, ct, None], op=OP.mult)
                    nc.vector.scalar_tensor_tensor(
                        out=tvec[:, ct:ct + 1], in0=cb[:, 0:1],
                        scalar=nsvec[:, ct:ct + 1], in1=gbi_sb[:, ct, None],
                        op0=OP.mult, op1=OP.add)
                # h = s_c * x + t_c  (in place over xb; also hq from xq)
                for nb in range(NB):
                    nsl = slice(nb * 512, (nb + 1) * 512)
                    for ct in range(CT):
                        nc.scalar.activation(out=h_sb[:, ct, nsl].bitcast(F32R),
                                             in_=xb_sb[:, ct, nsl],
                                             func=AF.Identity,
                                             bias=tvec[:, ct:ct + 1],
                                             scale=svec[:, ct:ct + 1])
                for ct in range(CT):
                    nc.vector.tensor_scalar(
                        out=hq_sb[:, ct, :].bitcast(F32R), in0=xq_sb[:, ct, :],
                        scalar1=svec[:, ct:ct + 1], scalar2=tvec[:, ct:ct + 1],
                        op0=OP.mult, op1=OP.add)
            # ================= Stage 2: K, V^T, Q, proj-bias =================
            fb_sb = stats.tile([P, CT], F32, tag="fb")  # wp @ bv + bp
            with (
                tc.tile_pool(name="ps2", bufs=2, space="PSUM") as ps2,
                tc.tile_pool(name="ps2k", bufs=3, space="PSUM") as ps2k,
                tc.tile_pool(name="ps2v", bufs=3, space="PSUM") as ps2v,
            ):
                for o in range(CT):
                    for nb in range(NB):
                        kps = ps2k.tile([P, 512], F32, tag="kps")
                        for ct in range(CT):
                            nc.tensor.matmul(
                                kps[:, :],
                                wk_sb[:, ct, o * P:(o + 1) * P],
                                r(h_sb[:, ct, nb * 512:(nb + 1) * 512]),
                                start=(ct == 0), stop=(ct == CT - 1))
                        nc.vector.tensor_copy(
                            k_sb[:, o, nb * 512:(nb + 1) * 512].bitcast(F32R),
                            kps[:, :])
                for jt in range(JT):
                    vps = ps2v.tile([P, C], F32, tag="vps")
                    for ct in range(CT):
                        nc.tensor.matmul(
                            vps[:, :],
                            r(h_sb[:, ct, jt * P:(jt + 1) * P]),
                            wv_sb[:, ct, :],
                            start=(ct == 0), stop=(ct == CT - 1))
                    nc.vector.tensor_copy(vt_sb[:, jt, :].bitcast(F32R), vps[:, :])
                for o in range(CT):
                    for ib in range(IC):
                        qps = ps2.tile([P, 512], F32, tag="qps")
                        for ct in range(CT):
                            nc.tensor.matmul(
                                qps[:, :],
                                wq_sb[:, ct, o * P:(o + 1) * P],
                                r(hq_sb[:, ct, ib * 512:(ib + 1) * 512]),
                                start=(ct == 0), stop=(ct == CT - 1))
                        nc.vector.tensor_scalar_add(
                            out=q_sb[:, o, ib * 512:(ib + 1) * 512].bitcast(F32R),
                            in0=qps[:, :], scalar1=bq_sb[:, o, None])
                    fps = ps2.tile([P, 2], F32, tag="qps")
                    for ct in range(CT):
                        nc.tensor.matmul(fps[:, :],
                                         wp_sb[:, ct, o * P:(o + 1) * P],
                                         bv_sb[:, ct, :],
                                         start=(ct == 0), stop=(ct == CT - 1))
                    nc.vector.tensor_tensor(out=fb_sb[:, o:o + 1], in0=fps[:, 0:1],
                                            in1=bp_sb[:, o, None], op=OP.add)

            # ================= Stage 3: attention per 512-query chunk =========
            with (
                tc.tile_pool(name="psA", bufs=1, space="PSUM") as psA,
                tc.tile_pool(name="psS", bufs=3, space="PSUM") as psS,
                tc.tile_pool(name="psM", bufs=1, space="PSUM") as psM,
            ):
                for ic in range(IC):
                    isl = slice(ic * 512, (ic + 1) * 512)
                    a0ps = psA.tile([P, 512], F32, tag="A0")
                    a1ps = psA.tile([P, 512], F32, tag="A1")
                    rsps = psA.tile([P, 512] if RS_PACK else [1, 512], F32,
                                    tag="rs")
                    if RS_PACK:
                        # zero the bank so only the 4 accumulator rows carry
                        # data; lets the end-of-loop combine be one wide copy
                        nc.vector.memset(rsps[:, :], 0.0)
                    aps = (a0ps, a1ps)
                    # software pipeline: S/exp of tile jt overlaps A-matmuls of
                    # tile jt-3 (exp latency fully hidden)
                    pts = [None] * JT
                    for jt in range(JT):
                        sps = psS.tile([P, 512], F32, tag="sps")
                        for o in range(CT):
                            nc.tensor.matmul(
                                sps[:, :],
                                r(k_sb[:, o, jt * P:(jt + 1) * P]),
                                r(q_sb[:, o, isl]),
                                start=(o == 0), stop=(o == CT - 1))
                        if dbg and ic == 0 and jt == 0:
                            dbg_s_sb = data.tile([P, 512], F32, tag="dbgs")
                            nc.vector.tensor_copy(dbg_s_sb[:, :], sps[:, :])
                            nc.sync.dma_start(out=dbg_s[:], in_=dbg_s_sb[:, :])
                        pt = ptiles.tile([P, 512], F32, tag="pt")
                        nc.scalar.activation(out=pt[:, :].bitcast(F32R), in_=sps[:, :],
                                             func=AF.Exp, bias=0.0, scale=SCALE)
                        pts[jt] = pt

                        def a_mms(j):
                            for ct in range(CT):
                                nc.tensor.matmul(
                                    aps[ct][:, :],
                                    r(vt_sb[:, j, ct * P:(ct + 1) * P]),
                                    r(pts[j][:, :]),
                                    start=(j == 0), stop=(j == JT - 1))
                            if RS_PACK:
                                if j % 4 == 3:
                                    # 4 back-to-back M=1 matmuls in distinct
                                    # column groups -> concurrent on the PE
                                    for k in range(4):
                                        jj = j - 3 + k
                                        nc.tensor.matmul(
                                            rsps[32 * k:32 * k + 1, :],
                                            r(ones_sb[:, :]),
                                            r(pts[jj][:, :]),
                                            start=(jj < 4), stop=(jj >= JT - 4),
                                            tile_position=(0, 32 * k),
                                            skip_group_check=True)
                            else:
                                nc.tensor.matmul(rsps[:, :], r(ones_sb[:, :]),
                                                 r(pts[j][:, :]),
                                                 start=(j == 0), stop=(j == JT - 1))

                        if jt > 2:
                            a_mms(jt - 3)
                    a_mms(JT - 3)
                    a_mms(JT - 2)
                    a_mms(JT - 1)
                    # Tail chain (rowsum combine -> recip -> broadcast) is
                    # the critical path at the end: emit it ahead of the as
                    # copies so it wins the DVE/PE queue slots.
                    if RS_PACK:
                        # rowsum = sum of the 4 packed partial rows: one wide
                        # copy of the zero-padded bank, one ones-contraction
                        rsc = astiles.tile([P, 512], F32, tag="rsc")
                        nc.vector.tensor_copy(rsc[:, :].bitcast(F32R),
                                              rsps[:, :])
                        nc.tensor.matmul(rsps[0:1, :], r(ones_sb[:, :]),
                                         r(rsc[:, :]),
                                         start=True, stop=True,
                                         skip_group_check=True)
                    recip = stats.tile([1, 512], F32, tag="recip")
                    nc.vector.reciprocal(out=recip[:, :],
                                         in_=rsps[0:1, :])
                    rb_sb = astiles.tile([P, 512], F32, tag="rbs")
                    nc.gpsimd.partition_broadcast(rb_sb[:, :], recip[:, :])
                    # Normalization by 1/rowsum is applied AFTER the projection
                    # (it commutes with the channel contraction), so the proj
                    # matmuls start as soon as A stops.
                    as_sb = astiles.tile([P, CT, 512], F32, tag="as")
                    for ct in range(CT):
                        nc.vector.tensor_copy(as_sb[:, ct, :].bitcast(F32R),
                                              aps[ct][:, :])
                    # projection; then out = proj*rb + (fbias + residual)
                    pps0 = psM.tile([P, 512], F32, tag="pp0")
                    pps1 = psM.tile([P, 512], F32, tag="pp1")
                    pps = (pps0, pps1)
                    for ct in range(CT):
                        for o in range(CT):
                            nc.tensor.matmul(
                                pps[o][:, :],
                                wp_sb[:, ct, o * P:(o + 1) * P],
                                r(as_sb[:, ct, :]),
                                start=(ct == 0), stop=(ct == CT - 1))
                    for o in range(CT):
                        nc.vector.tensor_tensor(
                            out=out_sb[:, o, isl], in0=pps[o][:, :],
                            in1=rb_sb[:, :], op=OP.mult)
                        nc.vector.scalar_tensor_tensor(
                            out=out_sb[:, o, isl], in0=out_sb[:, o, isl],
                            scalar=fb_sb[:, o:o + 1], in1=xq_sb[:, o, isl],
                            op0=OP.add, op1=OP.add)
                        nc.sync.dma_start(out=out_d[o, :, isl],
                                          in_=out_sb[:, o, isl])

            if dbg:
                for ct in range(CT):
                    nc.sync.dma_start(out=dbg_h[ct], in_=h_sb[:, ct, :])
                    nc.sync.dma_start(out=dbg_k[ct], in_=k_sb[:, ct, :])
                    nc.sync.dma_start(out=dbg_q[ct], in_=q_sb[:, ct, :])
                for jt in range(JT):
                    nc.sync.dma_start(out=dbg_vt[jt], in_=vt_sb[:, jt, :])

    nc.compile()
    return nc


_PROGRAM = None


def _get_program():
    global _PROGRAM
    if _PROGRAM is None:
        _PROGRAM = build_program()
    return _PROGRAM


def _in_maps(x):
    raise NotImplementedError  # replaced below; kept for clarity


def make_in_maps(x, gn_scale, gn_bias, wq, bq, wk, bk, wv, bv, wp, bp):
    x2 = np.ascontiguousarray(np.asarray(x, np.float32).reshape(B, C, N))
    cidx = np.arange(C)
    G_full = (cidx[:, None] // GSIZE == np.arange(NGROUPS)[None, :]).astype(np.float32)
    # bn_stats already averages over the free dim, so combining the GSIZE
    # per-channel (mean, E[x^2]) rows into a group stat divides by GSIZE only.
    csm = np.zeros((C, NGROUPS + 5), np.float32)
    csm[:, :NGROUPS] = G_full / GSIZE
    csm[:, NGROUPS + 0] = np.asarray(bq, np.float32)
    csm[:, NGROUPS + 1] = np.asarray(bp, np.float32)
    csm[:, NGROUPS + 2] = np.asarray(gn_scale, np.float32)
    csm[:, NGROUPS + 3] = np.asarray(gn_bias, np.float32)
    csm[:, NGROUPS + 4] = -np.asarray(gn_scale, np.float32)
    csm = np.ascontiguousarray(csm.reshape(CT, P, NGROUPS + 5))
    GT = np.ascontiguousarray(G_full.T)  # [32, 256]

    def wT(wm):
        return np.ascontiguousarray(np.asarray(wm, np.float32).T.reshape(CT, P, C))

    def col(v):
        return np.ascontiguousarray(np.asarray(v, np.float32).reshape(CT, P, 1))

    def col2(v):
        a = np.zeros((C, 2), np.float32)
        a[:, 0] = np.asarray(v, np.float32)
        return np.ascontiguousarray(a.reshape(CT, P, 2))

    shared = {
        "wqT": wT(wq), "wkT": wT(wk), "wvT": wT(wv), "wpT": wT(wp),
        "bv": col2(bv), "csm": csm, "GT": GT,
    }
    in_maps = []
    for core in range(8):
        bi, ci = divmod(core, 4)
        xb = np.ascontiguousarray(x2[bi].reshape(CT, P, N))
        xq = np.ascontiguousarray(
            x2[bi][:, ci * NQ:(ci + 1) * NQ].reshape(CT, P, NQ))
        in_maps.append(dict(shared, xb=xb, xq=xq))
    return in_maps


def run(in_maps, **kwargs):
    nc = _get_program()
    return run_bass_kernel_spmd(nc, in_maps, core_ids=list(range(8)), **kwargs)


def kernel(x, gn_scale, gn_bias, wq, bq, wk, bk, wv, bv, wp, bp):
    in_maps = make_in_maps(x, gn_scale, gn_bias, wq, bq, wk, bk, wv, bv, wp, bp)
    res = run(in_maps)
    out = np.empty((B, C, N), np.float32)
    for core in range(8):
        bi, ci = divmod(core, 4)
        out[bi][:, ci * NQ:(ci + 1) * NQ] = (
            res.results[core]["out"].reshape(C, NQ))
    return out.reshape(B, C, T, H, W)


if __name__ == "__main__":
    rng = np.random.default_rng(0)
    x = rng.standard_normal((B, C, T, H, W), dtype=np.float32)
    args = dict(
        x=x,
        gn_scale=np.ones(C, np.float32), gn_bias=np.zeros(C, np.float32),
        wq=rng.standard_normal((C, C), dtype=np.float32) / 16,
        bq=rng.standard_normal(C, dtype=np.float32) * 0.01,
        wk=rng.standard_normal((C, C), dtype=np.float32) / 16,
        bk=rng.standard_normal(C, dtype=np.float32) * 0.01,
        wv=rng.standard_normal((C, C), dtype=np.float32) / 16,
        bv=rng.standard_normal(C, dtype=np.float32) * 0.01,
        wp=rng.standard_normal((C, C), dtype=np.float32) / 16,
        bp=rng.standard_normal(C, dtype=np.float32) * 0.01,
    )
    out = kernel(**args)
    print("kernel ran, out shape", out.shape, "mean", float(out.mean()))


# revision 46
# speedup vs baseline: 1.0515x; 1.0515x over previous
"""NonLocalBlock (GroupNorm + 4096-token self-attention + proj + residual) on 8 TRN2 cores.

Sharding: core = (batch b in {0,1}, query-chunk q in {0..3}); each core holds its
batch's full x (needed for GN stats and K/V over all tokens) and computes the
output for its 1024-token query chunk. No collectives needed.

Math notes (exact reductions of the reference):
  - bk drops out: adding a per-j-constant... no -- adding k-bias shifts every
    logit of row i by q_i . bk, constant in j -> softmax invariant.
  - bv folds into the projection bias: softmax rows sum to 1, so
    proj(A + bv) = proj(A) + wp @ bv.
  - Normalization by the softmax row-sum commutes with the V- and P-matmuls,
    so we divide once on the small [c, i] result instead of the [i, j] matrix.
"""

import sys

for _p in ("/opt/trn_rl_repo",):
    if _p not in sys.path:
        sys.path.insert(0, _p)

import numpy as np

import concourse.bacc as bacc
import concourse.bass as bass
import concourse.tile as tile
from concourse import mybir
from concourse.bass_utils import run_bass_kernel_spmd

F32 = mybir.dt.float32
F32R = mybir.dt.float32r
AF = mybir.ActivationFunctionType
OP = mybir.AluOpType

B, C, T, H, W = 2, 256, 4, 32, 32
N = T * H * W            # 4096 tokens
NQ = N // 4              # 1024 query tokens per core
P = 128                  # partitions
CT = C // P              # 2 channel tiles
JT = N // P              # 32 key tiles of 128
NB = N // 512            # 8 key blocks of 512
IC = NQ // 512           # 2 query sub-chunks of 512
NGROUPS = 32
GSIZE = C // NGROUPS     # 8 channels per group
EPS = 1e-6
SCALE = C ** (-0.5)      # 1/16
# Pack the M=1 rowsum matmuls 4-at-a-time into disjoint PE column groups
# (tile_position) so they run concurrently -- each costs N cycles otherwise.
RS_PACK = False


def r(ap):
    """View an fp32 AP as float32r for full-rate PE matmuls (moving dim >= 256)."""
    return ap.bitcast(F32R)


def build_program(dbg=False):
    nc = bacc.Bacc("TRN2", target_bir_lowering=False, debug=False, num_devices=8)

    # ---- DRAM parameters (per core) ----
    xb_d = nc.declare_dram_parameter("xb", [CT, P, N], F32, isOutput=False)
    xq_d = nc.declare_dram_parameter("xq", [CT, P, NQ], F32, isOutput=False)
    wqT_d = nc.declare_dram_parameter("wqT", [CT, P, C], F32R, isOutput=False)
    wkT_d = nc.declare_dram_parameter("wkT", [CT, P, C], F32R, isOutput=False)
    wvT_d = nc.declare_dram_parameter("wvT", [CT, P, C], F32R, isOutput=False)
    wpT_d = nc.declare_dram_parameter("wpT", [CT, P, C], F32R, isOutput=False)
    # Packed small constants, one DMA: cols [0:32]=G group-indicator/GSIZE,
    # 32=bq, 33=bp, 34=gn_scale, 35=gn_bias.
    csm_d = nc.declare_dram_parameter("csm", [CT, P, NGROUPS + 5], F32,
                                      isOutput=False)
    bv_d = nc.declare_dram_parameter("bv", [CT, P, 2], F32R, isOutput=False)
    # GT[g, c] = (c//GSIZE == g)  (broadcast group stat back to channels)
    GT_d = nc.declare_dram_parameter("GT", [NGROUPS, C], F32, isOutput=False)
    out_d = nc.declare_dram_parameter("out", [CT, P, NQ], F32, isOutput=True)
    if dbg:
        dbg_h = nc.declare_dram_parameter("dbg_h", [CT, P, N], F32, isOutput=True)
        dbg_k = nc.declare_dram_parameter("dbg_k", [CT, P, N], F32, isOutput=True)
        dbg_vt = nc.declare_dram_parameter("dbg_vt", [JT, P, C], F32, isOutput=True)
        dbg_q = nc.declare_dram_parameter("dbg_q", [CT, P, NQ], F32, isOutput=True)
        dbg_s = nc.declare_dram_parameter("dbg_s", [P, 512], F32, isOutput=True)

    with tile.TileContext(nc) as tc:
        with (
            nc.allow_low_precision(reason="float32r rounding for full-rate PE"),
            tc.tile_pool(name="consts", bufs=1) as consts,
            tc.tile_pool(name="data", bufs=1) as data,
            tc.tile_pool(name="stats", bufs=1) as stats,
            tc.tile_pool(name="ptiles", bufs=8) as ptiles,
            tc.tile_pool(name="astiles", bufs=2) as astiles,
        ):
            # ---- input DMAs, one queue, explicit order by first-use time.
            # The ~330GB/s DMA pipe is the head bottleneck: small consts + wk
            # first (they gate the first PE ops), then the 4MB xb stream that
            # gates GN stats, then tensors needed progressively later.
            csm_sb = consts.tile([P, CT, NGROUPS + 5], F32, tag="csm")
            nc.sync.dma_start(out=csm_sb[:, :, :],
                              in_=csm_d.rearrange("ct p k -> p ct k"))
            G_sb = csm_sb[:, :, 0:NGROUPS]
            bq_sb = csm_sb[:, :, NGROUPS + 0]
            bp_sb = csm_sb[:, :, NGROUPS + 1]
            gsc_sb = csm_sb[:, :, NGROUPS + 2]
            gbi_sb = csm_sb[:, :, NGROUPS + 3]
            ngsc_sb = csm_sb[:, :, NGROUPS + 4]
            GT_sb = consts.tile([NGROUPS, C], F32, tag="GT")
            nc.sync.dma_start(out=GT_sb[:, :], in_=GT_d[:])
            # xb right behind the tiny stat constants: bn_stats consume chunks
            # at DMA rate, so the stats pipeline drains right after the last
            # chunk; everything else arrives just-in-time behind it.
            xb_sb = data.tile([P, CT, N], F32, tag="xb")      # raw x (stage 1 only)
            xq_sb = data.tile([P, CT, NQ], F32, tag="xq")
            for nb in range(NB):
                nsl = slice(nb * 512, (nb + 1) * 512)
                for ct in range(CT):
                    nc.sync.dma_start(out=xb_sb[:, ct, nsl], in_=xb_d[ct, :, nsl])
            wq_sb = consts.tile([P, CT, C], F32R, tag="wq")
            wk_sb = consts.tile([P, CT, C], F32R, tag="wk")
            wv_sb = consts.tile([P, CT, C], F32R, tag="wv")
            wp_sb = consts.tile([P, CT, C], F32R, tag="wp")
            nc.sync.dma_start(out=wk_sb[:, :, :],
                              in_=wkT_d.rearrange("ct p o -> p ct o"))
            nc.sync.dma_start(out=xq_sb[:, :, :],
                              in_=xq_d.rearrange("ct p i -> p ct i"))
            nc.sync.dma_start(out=wv_sb[:, :, :],
                              in_=wvT_d.rearrange("ct p o -> p ct o"))
            nc.sync.dma_start(out=wq_sb[:, :, :],
                              in_=wqT_d.rearrange("ct p o -> p ct o"))
            bv_sb = consts.tile([P, CT, 2], F32R, tag="bv")
            nc.sync.dma_start(out=bv_sb[:, :, :],
                              in_=bv_d.rearrange("ct p k -> p ct k"))
            nc.sync.dma_start(out=wp_sb[:, :, :],
                              in_=wpT_d.rearrange("ct p o -> p ct o"))
            ones_f = consts.tile([P, 1], F32, tag="ones_f")
            nc.vector.memset(ones_f[:, :], 1.0)
            ones_sb = consts.tile([P, 1], F32, tag="ones")
            nc.vector.tensor_copy(ones_sb[:, :].bitcast(F32R), ones_f[:, :])
            epsg_sb = consts.tile([NGROUPS, 1], F32, tag="epsg")
            nc.vector.memset(epsg_sb[:, :], EPS)

            # ---- big SBUF tensors ----
            h_sb = data.tile([P, CT, N], F32, tag="h")        # GN output
            hq_sb = data.tile([P, CT, NQ], F32, tag="hq")
            k_sb = data.tile([P, CT, N], F32, tag="k")        # K[o, j]
            # vt reuses xb's slot (same tag/size): xb is dead once h is built
            vt_sb = data.tile([P, JT, C], F32, tag="xb")      # V^T[j, o]
            q_sb = data.tile([P, CT, NQ], F32, tag="q")       # Q[o, i]
            out_sb = data.tile([P, CT, NQ], F32, tag="out")

            # ================= Stage 1: GroupNorm =================
            with tc.tile_pool(name="ps1", bufs=1, space="PSUM") as ps1:
                # per-channel mean/var over the 4096 free positions
                bst = stats.tile([P, CT, NB, 6], F32, tag="bst")
                mv = stats.tile([P, CT, 2], F32, tag="mv")
                mst = stats.tile([P, CT, 2], F32, tag="mst")   # (mean_c, E[x^2]_c)
                # nb-major to match DMA chunk arrival order (DVE is in-order)
                for nb in range(NB):
                    for ct in range(CT):
                        nc.vector.bn_stats(
                            out=bst[:, ct, nb, :],
                            in_=xb_sb[:, ct, nb * 512:(nb + 1) * 512],
                        )
                for ct in range(CT):
                    nc.vector.bn_aggr(out=mv[:, ct, :], in_=bst[:, ct, :, :])
                    nc.vector.tensor_copy(mst[:, ct, 0:1], mv[:, ct, 0:1])
                    # E[x^2] = var + mean^2
                    nc.vector.tensor_tensor(
                        out=mst[:, ct, 1:2], in0=mv[:, ct, 0:1],
                        in1=mv[:, ct, 0:1], op=OP.mult)
                    nc.vector.tensor_tensor(
                        out=mst[:, ct, 1:2], in0=mst[:, ct, 1:2],
                        in1=mv[:, ct, 1:2], op=OP.add)
                # group-sum across partitions: [g, (mean, Ex2)]
                gps = ps1.tile([NGROUPS, 2], F32, tag="gps")
                for ct in range(CT):
                    nc.tensor.matmul(gps[:, :], G_sb[:, ct, :], mst[:, ct, :],
                                     start=(ct == 0), stop=(ct == CT - 1))
                gmv = stats.tile([NGROUPS, 2], F32, tag="gmv")
                nc.vector.tensor_copy(gmv[:, :], gps[:, :])
                gtmp = stats.tile([NGROUPS, 1], F32, tag="gtmp")
                gvec = stats.tile([NGROUPS, 2], F32, tag="gvec")  # (mean_g, rstd_g)
                nc.vector.tensor_copy(gvec[:, 0:1], gmv[:, 0:1])
                # -var = mean^2 - E[x^2]; sqrt(var+eps) via scale=-1
                nc.vector.scalar_tensor_tensor(
                    out=gtmp, in0=gmv[:, 0:1], scalar=gmv[:, 0:1],
                    in1=gmv[:, 1:2], op0=OP.mult, op1=OP.subtract)
                nc.scalar.activation(out=gtmp, in_=gtmp, func=AF.Sqrt,
                                     bias=epsg_sb[:, :], scale=-1.0)
                nc.vector.reciprocal(out=gvec[:, 1:2], in_=gtmp)  # rstd_g
                # broadcast back to channels: cb[c, (mean, rstd)]
                svec = stats.tile([P, CT], F32, tag="svec")
                nsvec = stats.tile([P, CT], F32, tag="nsvec")
                tvec = stats.tile([P, CT], F32, tag="tvec")
                for ct in range(CT):
                    cps = ps1.tile([P, 2], F32, tag="cps")
                    nc.tensor.matmul(cps[:, :], GT_sb[:, ct * P:(ct + 1) * P],
                                     gvec[:, :], start=True, stop=True)
                    cb = stats.tile([P, 2], F32, tag="cb")
                    nc.vector.tensor_copy(cb[:, :], cps[:, :])
                    # s_c = rstd_c * gn_scale_c ; t_c = gn_bias_c + mean_c*(-s_c)
                    nc.vector.tensor_tensor(out=svec[:, ct:ct + 1], in0=cb[:, 1:2],
                                            in1=gsc_sb[:, ct, None], op=OP.mult)
                    nc.vector.tensor_tensor(out=nsvec[:, ct:ct + 1], in0=cb[:, 1:2],
                                            in1=ngsc_sb[:, ct, None], op=OP.mult)
                    nc.vector.scalar_tensor_tensor(
                        out=tvec[:, ct:ct + 1], in0=cb[:, 0:1],
                        scalar=nsvec[:, ct:ct + 1], in1=gbi_sb[:, ct, None],
                        op0=OP.mult, op1=OP.add)
                # h = s_c * x + t_c  (in place over xb; also hq from xq)
                for nb in range(NB):
                    nsl = slice(nb * 512, (nb + 1) * 512)
                    for ct in range(CT):
                        nc.scalar.activation(out=h_sb[:, ct, nsl].bitcast(F32R),
                                             in_=xb_sb[:, ct, nsl],
                                             func=AF.Identity,
                                             bias=tvec[:, ct:ct + 1],
                                             scale=svec[:, ct:ct + 1])
                for ct in range(CT):
                    nc.vector.tensor_scalar(
                        out=hq_sb[:, ct, :].bitcast(F32R), in0=xq_sb[:, ct, :],
                        scalar1=svec[:, ct:ct + 1], scalar2=tvec[:, ct:ct + 1],
                        op0=OP.mult, op1=OP.add)
            # ================= Stage 2: K, V^T, Q, proj-bias =================
            fb_sb = stats.tile([P, CT], F32, tag="fb")  # wp @ bv + bp
            with (
                tc.tile_pool(name="ps2", bufs=2, space="PSUM") as ps2,
                tc.tile_pool(name="ps2k", bufs=3, space="PSUM") as ps2k,
            ):
                def q_mms(o, ib):
                    qps = ps2.tile([P, 512], F32, tag="qps")
                    for ct in range(CT):
                        nc.tensor.matmul(
                            qps[:, :],
                            wq_sb[:, ct, o * P:(o + 1) * P],
                            r(hq_sb[:, ct, ib * 512:(ib + 1) * 512]),
                            start=(ct == 0), stop=(ct == CT - 1))
                    nc.scalar.activation(
                        out=q_sb[:, o, ib * 512:(ib + 1) * 512].bitcast(F32R),
                        in_=qps[:, :], func=AF.Identity,
                        bias=bq_sb[:, o, None], scale=1.0)

                # nb-major: K, V^T, Q interleaved along h-chunk readiness
                for nb in range(NB):
                    for o in range(CT):
                        kps = ps2k.tile([P, 512], F32, tag="kps")
                        for ct in range(CT):
                            nc.tensor.matmul(
                                kps[:, :],
                                wk_sb[:, ct, o * P:(o + 1) * P],
                                r(h_sb[:, ct, nb * 512:(nb + 1) * 512]),
                                start=(ct == 0), stop=(ct == CT - 1))
                        nc.vector.tensor_copy(
                            k_sb[:, o, nb * 512:(nb + 1) * 512].bitcast(F32R),
                            kps[:, :])
                    if nb == 0:
                        for o in range(CT):
                            for ib in range(IC):
                                q_mms(o, ib)
                for o in range(CT):
                    fps = ps2.tile([P, 2], F32, tag="qps")
                    for ct in range(CT):
                        nc.tensor.matmul(fps[:, :],
                                         wp_sb[:, ct, o * P:(o + 1) * P],
                                         bv_sb[:, ct, :],
                                         start=(ct == 0), stop=(ct == CT - 1))
                    nc.vector.tensor_tensor(out=fb_sb[:, o:o + 1], in0=fps[:, 0:1],
                                            in1=bp_sb[:, o, None], op=OP.add)

            # ================= Stage 3: attention per 512-query chunk =========
            with (
                tc.tile_pool(name="psA", bufs=1, space="PSUM") as psA,
                tc.tile_pool(name="psS", bufs=3, space="PSUM") as psS,
                tc.tile_pool(name="psV", bufs=2, space="PSUM") as psV,
            ):
                def vt_mms(jt):
                    # V^T tile production, interleaved into the ic0 attention
                    # loop: fills PE stall slots and gives the PSUM->SBUF
                    # copies slack
                    vps = psV.tile([P, C], F32, tag="vps")
                    for ct in range(CT):
                        nc.tensor.matmul(
                            vps[:, :],
                            r(h_sb[:, ct, jt * P:(jt + 1) * P]),
                            wv_sb[:, ct, :],
                            start=(ct == 0), stop=(ct == CT - 1))
                    nc.vector.tensor_copy(vt_sb[:, jt, :].bitcast(F32R),
                                          vps[:, :])
                for ic in range(IC):
                    if ic == 0:
                        for jt in range(7):
                            vt_mms(jt)
                    isl = slice(ic * 512, (ic + 1) * 512)
                    a0ps = psA.tile([P, 512], F32, tag="A0")
                    a1ps = psA.tile([P, 512], F32, tag="A1")
                    rsps = psA.tile([P, 512] if RS_PACK else [1, 512], F32,
                                    tag="rs")
                    if RS_PACK:
                        # zero the bank so only the 4 accumulator rows carry
                        # data; lets the end-of-loop combine be one wide copy
                        nc.vector.memset(rsps[:, :], 0.0)
                    aps = (a0ps, a1ps)
                    # software pipeline: S/exp of tile jt overlaps A-matmuls of
                    # tile jt-3 (exp latency fully hidden)
                    pts = [None] * JT
                    for jt in range(JT):
                        sps = psS.tile([P, 512], F32, tag="sps")
                        for o in range(CT):
                            nc.tensor.matmul(
                                sps[:, :],
                                r(k_sb[:, o, jt * P:(jt + 1) * P]),
                                r(q_sb[:, o, isl]),
                                start=(o == 0), stop=(o == CT - 1))
                        if dbg and ic == 0 and jt == 0:
                            dbg_s_sb = data.tile([P, 512], F32, tag="dbgs")
                            nc.vector.tensor_copy(dbg_s_sb[:, :], sps[:, :])
                            nc.sync.dma_start(out=dbg_s[:], in_=dbg_s_sb[:, :])
                        pt = ptiles.tile([P, 512], F32, tag="pt")
                        nc.scalar.activation(out=pt[:, :].bitcast(F32R), in_=sps[:, :],
                                             func=AF.Exp, bias=0.0, scale=SCALE)
                        pts[jt] = pt
                        if ic == 0 and jt + 7 < JT:
                            vt_mms(jt + 7)

                        def a_mms_rs(j):
                            if RS_PACK:
                                if j % 4 == 3:
                                    # 4 back-to-back M=1 matmuls in distinct
                                    # column groups -> concurrent on the PE
                                    for k in range(4):
                                        jj = j - 3 + k
                                        nc.tensor.matmul(
                                            rsps[32 * k:32 * k + 1, :],
                                            r(ones_sb[:, :]),
                                            r(pts[jj][:, :]),
                                            start=(jj < 4), stop=(jj >= JT - 4),
                                            tile_position=(0, 32 * k),
                                            skip_group_check=True)
                            else:
                                nc.tensor.matmul(rsps[:, :], r(ones_sb[:, :]),
                                                 r(pts[j][:, :]),
                                                 start=(j == 0), stop=(j == JT - 1))

                        def a_mms2(j):
                            for ct in range(CT):
                                nc.tensor.matmul(
                                    aps[ct][:, :],
                                    r(vt_sb[:, j, ct * P:(ct + 1) * P]),
                                    r(pts[j][:, :]),
                                    start=(j == 0), stop=(j == JT - 1))
                            a_mms_rs(j)

                        if jt > 2:
                            a_mms2(jt - 3)
                    a_mms2(JT - 3)
                    a_mms2(JT - 2)
                    a_mms2(JT - 1)
                    # Tail chain (rowsum combine -> recip -> broadcast) is
                    # the critical path at the end: emit it ahead of the as
                    # copies so it wins the DVE/PE queue slots.
                    if RS_PACK:
                        # rowsum = sum of the 4 packed partial rows: one wide
                        # copy of the zero-padded bank, one ones-contraction
                        rsc = astiles.tile([P, 512], F32, tag="rsc")
                        nc.vector.tensor_copy(rsc[:, :].bitcast(F32R),
                                              rsps[:, :])
                        nc.tensor.matmul(rsps[0:1, :], r(ones_sb[:, :]),
                                         r(rsc[:, :]),
                                         start=True, stop=True,
                                         skip_group_check=True)
                    recip = stats.tile([1, 512], F32, tag="recip")
                    nc.vector.reciprocal(out=recip[:, :],
                                         in_=rsps[0:1, :])
                    rb_sb = astiles.tile([P, 512], F32, tag="rbs")
                    nc.gpsimd.partition_broadcast(rb_sb[:, :], recip[:, :])
                    # Normalization by 1/rowsum is applied AFTER the projection
                    # (it commutes with the channel contraction), so the proj
                    # matmuls start as soon as A stops.
                    as_sb = astiles.tile([P, CT, 512], F32, tag="as")
                    for ct in range(CT):
                        nc.vector.tensor_copy(as_sb[:, ct, :].bitcast(F32R),
                                              aps[ct][:, :])
                    # projection; then out = proj*rb + (fbias + residual)
                    pps0 = psS.tile([P, 512], F32, tag="sps")
                    pps1 = psS.tile([P, 512], F32, tag="sps")
                    pps = (pps0, pps1)
                    for ct in range(CT):
                        for o in range(CT):
                            nc.tensor.matmul(
                                pps[o][:, :],
                                wp_sb[:, ct, o * P:(o + 1) * P],
                                r(as_sb[:, ct, :]),
                                start=(ct == 0), stop=(ct == CT - 1))
                    for o in range(CT):
                        nc.vector.tensor_tensor(
                            out=out_sb[:, o, isl], in0=pps[o][:, :],
                            in1=rb_sb[:, :], op=OP.mult)
                        nc.vector.scalar_tensor_tensor(
                            out=out_sb[:, o, isl], in0=out_sb[:, o, isl],
                            scalar=fb_sb[:, o:o + 1], in1=xq_sb[:, o, isl],
                            op0=OP.add, op1=OP.add)
                        nc.sync.dma_start(out=out_d[o, :, isl],
                                          in_=out_sb[:, o, isl])

            if dbg:
                for ct in range(CT):
                    nc.sync.dma_start(out=dbg_h[ct], in_=h_sb[:, ct, :])
                    nc.sync.dma_start(out=dbg_k[ct], in_=k_sb[:, ct, :])
                    nc.sync.dma_start(out=dbg_q[ct], in_=q_sb[:, ct, :])
                for jt in range(JT):
                    nc.sync.dma_start(out=dbg_vt[jt], in_=vt_sb[:, jt, :])

    nc.compile()
    return nc


_PROGRAM = None


def _get_program():
    global _PROGRAM
    if _PROGRAM is None:
        _PROGRAM = build_program()
    return _PROGRAM


def _in_maps(x):
    raise NotImplementedError  # replaced below; kept for clarity


def make_in_maps(x, gn_scale, gn_bias, wq, bq, wk, bk, wv, bv, wp, bp):
    x2 = np.ascontiguousarray(np.asarray(x, np.float32).reshape(B, C, N))
    cidx = np.arange(C)
    G_full = (cidx[:, None] // GSIZE == np.arange(NGROUPS)[None, :]).astype(np.float32)
    # bn_stats already averages over the free dim, so combining the GSIZE
    # per-channel (mean, E[x^2]) rows into a group stat divides by GSIZE only.
    csm = np.zeros((C, NGROUPS + 5), np.float32)
    csm[:, :NGROUPS] = G_full / GSIZE
    csm[:, NGROUPS + 0] = np.asarray(bq, np.float32)
    csm[:, NGROUPS + 1] = np.asarray(bp, np.float32)
    csm[:, NGROUPS + 2] = np.asarray(gn_scale, np.float32)
    csm[:, NGROUPS + 3] = np.asarray(gn_bias, np.float32)
    csm[:, NGROUPS + 4] = -np.asarray(gn_scale, np.float32)
    csm = np.ascontiguousarray(csm.reshape(CT, P, NGROUPS + 5))
    GT = np.ascontiguousarray(G_full.T)  # [32, 256]

    def wT(wm):
        return np.ascontiguousarray(np.asarray(wm, np.float32).T.reshape(CT, P, C))

    def col(v):
        return np.ascontiguousarray(np.asarray(v, np.float32).reshape(CT, P, 1))

    def col2(v):
        a = np.zeros((C, 2), np.float32)
        a[:, 0] = np.asarray(v, np.float32)
        return np.ascontiguousarray(a.reshape(CT, P, 2))

    shared = {
        "wqT": wT(wq), "wkT": wT(wk), "wvT": wT(wv), "wpT": wT(wp),
        "bv": col2(bv), "csm": csm, "GT": GT,
    }
    in_maps = []
    for core in range(8):
        bi, ci = divmod(core, 4)
        xb = np.ascontiguousarray(x2[bi].reshape(CT, P, N))
        xq = np.ascontiguousarray(
            x2[bi][:, ci * NQ:(ci + 1) * NQ].reshape(CT, P, NQ))
        in_maps.append(dict(shared, xb=xb, xq=xq))
    return in_maps


def run(in_maps, **kwargs):
    nc = _get_program()
    return run_bass_kernel_spmd(nc, in_maps, core_ids=list(range(8)), **kwargs)


def kernel(x, gn_scale, gn_bias, wq, bq, wk, bk, wv, bv, wp, bp):
    in_maps = make_in_maps(x, gn_scale, gn_bias, wq, bq, wk, bk, wv, bv, wp, bp)
    res = run(in_maps)
    out = np.empty((B, C, N), np.float32)
    for core in range(8):
        bi, ci = divmod(core, 4)
        out[bi][:, ci * NQ:(ci + 1) * NQ] = (
            res.results[core]["out"].reshape(C, NQ))
    return out.reshape(B, C, T, H, W)


if __name__ == "__main__":
    rng = np.random.default_rng(0)
    x = rng.standard_normal((B, C, T, H, W), dtype=np.float32)
    args = dict(
        x=x,
        gn_scale=np.ones(C, np.float32), gn_bias=np.zeros(C, np.float32),
        wq=rng.standard_normal((C, C), dtype=np.float32) / 16,
        bq=rng.standard_normal(C, dtype=np.float32) * 0.01,
        wk=rng.standard_normal((C, C), dtype=np.float32) / 16,
        bk=rng.standard_normal(C, dtype=np.float32) * 0.01,
        wv=rng.standard_normal((C, C), dtype=np.float32) / 16,
        bv=rng.standard_normal(C, dtype=np.float32) * 0.01,
        wp=rng.standard_normal((C, C), dtype=np.float32) / 16,
        bp=rng.standard_normal(C, dtype=np.float32) * 0.01,
    )
    out = kernel(**args)
    print("kernel ran, out shape", out.shape, "mean", float(out.mean()))


# revision 47
# speedup vs baseline: 1.0741x; 1.0215x over previous
"""NonLocalBlock (GroupNorm + 4096-token self-attention + proj + residual) on 8 TRN2 cores.

Sharding: core = (batch b in {0,1}, query-chunk q in {0..3}); each core holds its
batch's full x (needed for GN stats and K/V over all tokens) and computes the
output for its 1024-token query chunk. No collectives needed.

Math notes (exact reductions of the reference):
  - bk drops out: adding a per-j-constant... no -- adding k-bias shifts every
    logit of row i by q_i . bk, constant in j -> softmax invariant.
  - bv folds into the projection bias: softmax rows sum to 1, so
    proj(A + bv) = proj(A) + wp @ bv.
  - Normalization by the softmax row-sum commutes with the V- and P-matmuls,
    so we divide once on the small [c, i] result instead of the [i, j] matrix.
"""

import sys

for _p in ("/opt/trn_rl_repo",):
    if _p not in sys.path:
        sys.path.insert(0, _p)

import numpy as np

import concourse.bacc as bacc
import concourse.bass as bass
import concourse.tile as tile
from concourse import mybir
from concourse.bass_utils import run_bass_kernel_spmd

F32 = mybir.dt.float32
F32R = mybir.dt.float32r
AF = mybir.ActivationFunctionType
OP = mybir.AluOpType

B, C, T, H, W = 2, 256, 4, 32, 32
N = T * H * W            # 4096 tokens
NQ = N // 4              # 1024 query tokens per core
P = 128                  # partitions
CT = C // P              # 2 channel tiles
JT = N // P              # 32 key tiles of 128
NB = N // 512            # 8 key blocks of 512
IC = NQ // 512           # 2 query sub-chunks of 512
NGROUPS = 32
GSIZE = C // NGROUPS     # 8 channels per group
EPS = 1e-6
SCALE = C ** (-0.5)      # 1/16
# Pack the M=1 rowsum matmuls 4-at-a-time into disjoint PE column groups
# (tile_position) so they run concurrently -- each costs N cycles otherwise.
RS_PACK = False


def r(ap):
    """View an fp32 AP as float32r for full-rate PE matmuls (moving dim >= 256)."""
    return ap.bitcast(F32R)


def build_program(dbg=False):
    nc = bacc.Bacc("TRN2", target_bir_lowering=False, debug=False, num_devices=8)

    # ---- DRAM parameters (per core) ----
    xb_d = nc.declare_dram_parameter("xb", [CT, P, N], F32, isOutput=False)
    xq_d = nc.declare_dram_parameter("xq", [CT, P, NQ], F32, isOutput=False)
    wqT_d = nc.declare_dram_parameter("wqT", [CT, P, C], F32R, isOutput=False)
    wkT_d = nc.declare_dram_parameter("wkT", [CT, P, C], F32R, isOutput=False)
    wvT_d = nc.declare_dram_parameter("wvT", [CT, P, C], F32R, isOutput=False)
    wpT_d = nc.declare_dram_parameter("wpT", [CT, P, C], F32R, isOutput=False)
    # Packed small constants, one DMA: cols [0:32]=G group-indicator/GSIZE,
    # 32=bq, 33=bp, 34=gn_scale, 35=gn_bias.
    csm_d = nc.declare_dram_parameter("csm", [CT, P, NGROUPS + 5], F32,
                                      isOutput=False)
    bv_d = nc.declare_dram_parameter("bv", [CT, P, 2], F32R, isOutput=False)
    # GT[g, c] = (c//GSIZE == g)  (broadcast group stat back to channels)
    GT_d = nc.declare_dram_parameter("GT", [NGROUPS, C], F32, isOutput=False)
    out_d = nc.declare_dram_parameter("out", [CT, P, NQ], F32, isOutput=True)
    if dbg:
        dbg_h = nc.declare_dram_parameter("dbg_h", [CT, P, N], F32, isOutput=True)
        dbg_k = nc.declare_dram_parameter("dbg_k", [CT, P, N], F32, isOutput=True)
        dbg_vt = nc.declare_dram_parameter("dbg_vt", [JT, P, C], F32, isOutput=True)
        dbg_q = nc.declare_dram_parameter("dbg_q", [CT, P, NQ], F32, isOutput=True)
        dbg_s = nc.declare_dram_parameter("dbg_s", [P, 512], F32, isOutput=True)

    with tile.TileContext(nc) as tc:
        with (
            nc.allow_low_precision(reason="float32r rounding for full-rate PE"),
            tc.tile_pool(name="consts", bufs=1) as consts,
            tc.tile_pool(name="data", bufs=1) as data,
            tc.tile_pool(name="stats", bufs=1) as stats,
            tc.tile_pool(name="ptiles", bufs=8) as ptiles,
            tc.tile_pool(name="astiles", bufs=2) as astiles,
        ):
            # ---- input DMAs, one queue, explicit order by first-use time.
            # The ~330GB/s DMA pipe is the head bottleneck: small consts + wk
            # first (they gate the first PE ops), then the 4MB xb stream that
            # gates GN stats, then tensors needed progressively later.
            csm_sb = consts.tile([P, CT, NGROUPS + 5], F32, tag="csm")
            nc.sync.dma_start(out=csm_sb[:, :, :],
                              in_=csm_d.rearrange("ct p k -> p ct k"))
            G_sb = csm_sb[:, :, 0:NGROUPS]
            bq_sb = csm_sb[:, :, NGROUPS + 0]
            bp_sb = csm_sb[:, :, NGROUPS + 1]
            gsc_sb = csm_sb[:, :, NGROUPS + 2]
            gbi_sb = csm_sb[:, :, NGROUPS + 3]
            ngsc_sb = csm_sb[:, :, NGROUPS + 4]
            GT_sb = consts.tile([NGROUPS, C], F32, tag="GT")
            nc.sync.dma_start(out=GT_sb[:, :], in_=GT_d[:])
            # xb right behind the tiny stat constants: bn_stats consume chunks
            # at DMA rate, so the stats pipeline drains right after the last
            # chunk; everything else arrives just-in-time behind it.
            xb_sb = data.tile([P, CT, N], F32, tag="xb")      # raw x (stage 1 only)
            xq_sb = data.tile([P, CT, NQ], F32, tag="xq")
            for nb in range(NB):
                nsl = slice(nb * 512, (nb + 1) * 512)
                for ct in range(CT):
                    nc.sync.dma_start(out=xb_sb[:, ct, nsl], in_=xb_d[ct, :, nsl])
            wq_sb = consts.tile([P, CT, C], F32R, tag="wq")
            wk_sb = consts.tile([P, CT, C], F32R, tag="wk")
            wv_sb = consts.tile([P, CT, C], F32R, tag="wv")
            wp_sb = consts.tile([P, CT, C], F32R, tag="wp")
            nc.sync.dma_start(out=wk_sb[:, :, :],
                              in_=wkT_d.rearrange("ct p o -> p ct o"))
            nc.sync.dma_start(out=xq_sb[:, :, :],
                              in_=xq_d.rearrange("ct p i -> p ct i"))
            nc.sync.dma_start(out=wv_sb[:, :, :],
                              in_=wvT_d.rearrange("ct p o -> p ct o"))
            nc.sync.dma_start(out=wq_sb[:, :, :],
                              in_=wqT_d.rearrange("ct p o -> p ct o"))
            bv_sb = consts.tile([P, CT, 2], F32R, tag="bv")
            nc.sync.dma_start(out=bv_sb[:, :, :],
                              in_=bv_d.rearrange("ct p k -> p ct k"))
            nc.sync.dma_start(out=wp_sb[:, :, :],
                              in_=wpT_d.rearrange("ct p o -> p ct o"))
            ones_f = consts.tile([P, 1], F32, tag="ones_f")
            nc.vector.memset(ones_f[:, :], 1.0)
            ones_sb = consts.tile([P, 1], F32, tag="ones")
            nc.vector.tensor_copy(ones_sb[:, :].bitcast(F32R), ones_f[:, :])
            epsg_sb = consts.tile([NGROUPS, 1], F32, tag="epsg")
            nc.vector.memset(epsg_sb[:, :], EPS)

            # ---- big SBUF tensors ----
            h_sb = data.tile([P, CT, N], F32, tag="h")        # GN output
            hq_sb = data.tile([P, CT, NQ], F32, tag="hq")
            k_sb = data.tile([P, CT, N], F32, tag="k")        # K[o, j]
            # vt reuses xb's slot (same tag/size): xb is dead once h is built
            vt_sb = data.tile([P, JT, C], F32, tag="xb")      # V^T[j, o]
            q_sb = data.tile([P, CT, NQ], F32, tag="q")       # Q[o, i]
            out_sb = data.tile([P, CT, NQ], F32, tag="out")

            # ================= Stage 1: GroupNorm =================
            # Per-channel stats split across engines so they complete right
            # behind the xb DMA stream: DVE runs bn_stats on 5 chunks, ACT
            # accumulates raw (sum, sumsq) on the 3 middle ones.
            DVE_NBS = [0, 1, 2, 3, 7]
            ACT_NBS = [4, 5, 6]
            ND, NA = len(DVE_NBS), len(ACT_NBS)
            with tc.tile_pool(name="ps1", bufs=1, space="PSUM") as ps1:
                bst = stats.tile([P, CT, ND, 6], F32, tag="bst")
                mv = stats.tile([P, CT, 2], F32, tag="mv")
                sacc = stats.tile([P, CT, NA], F32, tag="sacc")
                qacc = stats.tile([P, CT, NA], F32, tag="qacc")
                scr = stats.tile([P, 512], F32, tag="scr")
                mst = stats.tile([P, CT, 2], F32, tag="mst")   # (mean_c, E[x^2]_c)
                stt1 = stats.tile([P, CT, 4], F32, tag="stt1")
                for nb in range(NB):
                    for ct in range(CT):
                        nsl = slice(nb * 512, (nb + 1) * 512)
                        if nb in DVE_NBS:
                            nc.vector.bn_stats(
                                out=bst[:, ct, DVE_NBS.index(nb), :],
                                in_=xb_sb[:, ct, nsl],
                            )
                        else:
                            ia = ACT_NBS.index(nb)
                            nc.scalar.activation(
                                out=scr[:, :], in_=xb_sb[:, ct, nsl],
                                func=AF.Identity,
                                accum_out=sacc[:, ct, ia:ia + 1])
                            nc.scalar.activation(
                                out=scr[:, :], in_=xb_sb[:, ct, nsl],
                                func=AF.Square,
                                accum_out=qacc[:, ct, ia:ia + 1])
                wd = ND * 512.0 / N   # weight of the bn_stats fraction
                for ct in range(CT):
                    nc.vector.bn_aggr(out=mv[:, ct, :], in_=bst[:, ct, :, :])
                    # mean_c = wd*m5 + sum_act/N
                    sact = stt1[:, ct, 0:1]
                    qact = stt1[:, ct, 1:2]
                    nc.vector.reduce_sum(out=sact, in_=sacc[:, ct, :],
                                         axis=mybir.AxisListType.X)
                    nc.vector.reduce_sum(out=qact, in_=qacc[:, ct, :],
                                         axis=mybir.AxisListType.X)
                    tm = stt1[:, ct, 2:3]
                    nc.vector.tensor_scalar_mul(out=tm, in0=mv[:, ct, 0:1],
                                                scalar1=wd)
                    nc.vector.scalar_tensor_tensor(
                        out=mst[:, ct, 0:1], in0=sact, scalar=1.0 / N,
                        in1=tm, op0=OP.mult, op1=OP.add)
                    # E[x^2]_c = wd*(v5 + m5^2) + sumsq_act/N
                    te = stt1[:, ct, 3:4]
                    nc.vector.tensor_tensor(out=te, in0=mv[:, ct, 0:1],
                                            in1=mv[:, ct, 0:1], op=OP.mult)
                    nc.vector.tensor_tensor(out=te, in0=te,
                                            in1=mv[:, ct, 1:2], op=OP.add)
                    nc.vector.tensor_scalar_mul(out=te, in0=te, scalar1=wd)
                    nc.vector.scalar_tensor_tensor(
                        out=mst[:, ct, 1:2], in0=qact, scalar=1.0 / N,
                        in1=te, op0=OP.mult, op1=OP.add)
                # group-sum across partitions: [g, (mean, Ex2)]
                gps = ps1.tile([NGROUPS, 2], F32, tag="gps")
                for ct in range(CT):
                    nc.tensor.matmul(gps[:, :], G_sb[:, ct, :], mst[:, ct, :],
                                     start=(ct == 0), stop=(ct == CT - 1))
                gmv = stats.tile([NGROUPS, 2], F32, tag="gmv")
                nc.vector.tensor_copy(gmv[:, :], gps[:, :])
                gtmp = stats.tile([NGROUPS, 1], F32, tag="gtmp")
                gvec = stats.tile([NGROUPS, 2], F32, tag="gvec")  # (mean_g, rstd_g)
                nc.vector.tensor_copy(gvec[:, 0:1], gmv[:, 0:1])
                # -var = mean^2 - E[x^2]; sqrt(var+eps) via scale=-1
                nc.vector.scalar_tensor_tensor(
                    out=gtmp, in0=gmv[:, 0:1], scalar=gmv[:, 0:1],
                    in1=gmv[:, 1:2], op0=OP.mult, op1=OP.subtract)
                nc.scalar.activation(out=gtmp, in_=gtmp, func=AF.Sqrt,
                                     bias=epsg_sb[:, :], scale=-1.0)
                nc.vector.reciprocal(out=gvec[:, 1:2], in_=gtmp)  # rstd_g
                # broadcast back to channels: cb[c, (mean, rstd)]
                svec = stats.tile([P, CT], F32, tag="svec")
                nsvec = stats.tile([P, CT], F32, tag="nsvec")
                tvec = stats.tile([P, CT], F32, tag="tvec")
                for ct in range(CT):
                    cps = ps1.tile([P, 2], F32, tag="cps")
                    nc.tensor.matmul(cps[:, :], GT_sb[:, ct * P:(ct + 1) * P],
                                     gvec[:, :], start=True, stop=True)
                    cb = stats.tile([P, 2], F32, tag="cb")
                    nc.vector.tensor_copy(cb[:, :], cps[:, :])
                    # s_c = rstd_c * gn_scale_c ; t_c = gn_bias_c + mean_c*(-s_c)
                    nc.vector.tensor_tensor(out=svec[:, ct:ct + 1], in0=cb[:, 1:2],
                                            in1=gsc_sb[:, ct, None], op=OP.mult)
                    nc.vector.tensor_tensor(out=nsvec[:, ct:ct + 1], in0=cb[:, 1:2],
                                            in1=ngsc_sb[:, ct, None], op=OP.mult)
                    nc.vector.scalar_tensor_tensor(
                        out=tvec[:, ct:ct + 1], in0=cb[:, 0:1],
                        scalar=nsvec[:, ct:ct + 1], in1=gbi_sb[:, ct, None],
                        op0=OP.mult, op1=OP.add)
                # h = s_c * x + t_c  (in place over xb; also hq from xq)
                for nb in range(NB):
                    nsl = slice(nb * 512, (nb + 1) * 512)
                    for ct in range(CT):
                        nc.scalar.activation(out=h_sb[:, ct, nsl].bitcast(F32R),
                                             in_=xb_sb[:, ct, nsl],
                                             func=AF.Identity,
                                             bias=tvec[:, ct:ct + 1],
                                             scale=svec[:, ct:ct + 1])
                for ct in range(CT):
                    nc.vector.tensor_scalar(
                        out=hq_sb[:, ct, :].bitcast(F32R), in0=xq_sb[:, ct, :],
                        scalar1=svec[:, ct:ct + 1], scalar2=tvec[:, ct:ct + 1],
                        op0=OP.mult, op1=OP.add)
            # ================= Stage 2: K, V^T, Q, proj-bias =================
            fb_sb = stats.tile([P, CT], F32, tag="fb")  # wp @ bv + bp
            with (
                tc.tile_pool(name="ps2", bufs=2, space="PSUM") as ps2,
                tc.tile_pool(name="ps2k", bufs=3, space="PSUM") as ps2k,
            ):
                def q_mms(o, ib):
                    qps = ps2.tile([P, 512], F32, tag="qps")
                    for ct in range(CT):
                        nc.tensor.matmul(
                            qps[:, :],
                            wq_sb[:, ct, o * P:(o + 1) * P],
                            r(hq_sb[:, ct, ib * 512:(ib + 1) * 512]),
                            start=(ct == 0), stop=(ct == CT - 1))
                    nc.scalar.activation(
                        out=q_sb[:, o, ib * 512:(ib + 1) * 512].bitcast(F32R),
                        in_=qps[:, :], func=AF.Identity,
                        bias=bq_sb[:, o, None], scale=1.0)

                # nb-major: K, V^T, Q interleaved along h-chunk readiness
                for nb in range(NB):
                    for o in range(CT):
                        kps = ps2k.tile([P, 512], F32, tag="kps")
                        for ct in range(CT):
                            nc.tensor.matmul(
                                kps[:, :],
                                wk_sb[:, ct, o * P:(o + 1) * P],
                                r(h_sb[:, ct, nb * 512:(nb + 1) * 512]),
                                start=(ct == 0), stop=(ct == CT - 1))
                        nc.vector.tensor_copy(
                            k_sb[:, o, nb * 512:(nb + 1) * 512].bitcast(F32R),
                            kps[:, :])
                    if nb == 0:
                        for o in range(CT):
                            for ib in range(IC):
                                q_mms(o, ib)
                for o in range(CT):
                    fps = ps2.tile([P, 2], F32, tag="qps")
                    for ct in range(CT):
                        nc.tensor.matmul(fps[:, :],
                                         wp_sb[:, ct, o * P:(o + 1) * P],
                                         bv_sb[:, ct, :],
                                         start=(ct == 0), stop=(ct == CT - 1))
                    nc.vector.tensor_tensor(out=fb_sb[:, o:o + 1], in0=fps[:, 0:1],
                                            in1=bp_sb[:, o, None], op=OP.add)

            # ================= Stage 3: attention per 512-query chunk =========
            with (
                tc.tile_pool(name="psA", bufs=1, space="PSUM") as psA,
                tc.tile_pool(name="psS", bufs=3, space="PSUM") as psS,
                tc.tile_pool(name="psV", bufs=2, space="PSUM") as psV,
            ):
                def vt_mms(jt):
                    # V^T tile production, interleaved into the ic0 attention
                    # loop: fills PE stall slots and gives the PSUM->SBUF
                    # copies slack
                    vps = psV.tile([P, C], F32, tag="vps")
                    for ct in range(CT):
                        nc.tensor.matmul(
                            vps[:, :],
                            r(h_sb[:, ct, jt * P:(jt + 1) * P]),
                            wv_sb[:, ct, :],
                            start=(ct == 0), stop=(ct == CT - 1))
                    nc.vector.tensor_copy(vt_sb[:, jt, :].bitcast(F32R),
                                          vps[:, :])
                for ic in range(IC):
                    if ic == 0:
                        for jt in range(7):
                            vt_mms(jt)
                    isl = slice(ic * 512, (ic + 1) * 512)
                    a0ps = psA.tile([P, 512], F32, tag="A0")
                    a1ps = psA.tile([P, 512], F32, tag="A1")
                    rsps = psA.tile([P, 512] if RS_PACK else [1, 512], F32,
                                    tag="rs")
                    if RS_PACK:
                        # zero the bank so only the 4 accumulator rows carry
                        # data; lets the end-of-loop combine be one wide copy
                        nc.vector.memset(rsps[:, :], 0.0)
                    aps = (a0ps, a1ps)
                    # software pipeline: S/exp of tile jt overlaps A-matmuls of
                    # tile jt-3 (exp latency fully hidden)
                    pts = [None] * JT
                    for jt in range(JT):
                        sps = psS.tile([P, 512], F32, tag="sps")
                        for o in range(CT):
                            nc.tensor.matmul(
                                sps[:, :],
                                r(k_sb[:, o, jt * P:(jt + 1) * P]),
                                r(q_sb[:, o, isl]),
                                start=(o == 0), stop=(o == CT - 1))
                        if dbg and ic == 0 and jt == 0:
                            dbg_s_sb = data.tile([P, 512], F32, tag="dbgs")
                            nc.vector.tensor_copy(dbg_s_sb[:, :], sps[:, :])
                            nc.sync.dma_start(out=dbg_s[:], in_=dbg_s_sb[:, :])
                        pt = ptiles.tile([P, 512], F32, tag="pt")
                        nc.scalar.activation(out=pt[:, :].bitcast(F32R), in_=sps[:, :],
                                             func=AF.Exp, bias=0.0, scale=SCALE)
                        pts[jt] = pt
                        if ic == 0 and jt + 7 < JT:
                            vt_mms(jt + 7)

                        def a_mms_rs(j):
                            if RS_PACK:
                                if j % 4 == 3:
                                    # 4 back-to-back M=1 matmuls in distinct
                                    # column groups -> concurrent on the PE
                                    for k in range(4):
                                        jj = j - 3 + k
                                        nc.tensor.matmul(
                                            rsps[32 * k:32 * k + 1, :],
                                            r(ones_sb[:, :]),
                                            r(pts[jj][:, :]),
                                            start=(jj < 4), stop=(jj >= JT - 4),
                                            tile_position=(0, 32 * k),
                                            skip_group_check=True)
                            else:
                                nc.tensor.matmul(rsps[:, :], r(ones_sb[:, :]),
                                                 r(pts[j][:, :]),
                                                 start=(j == 0), stop=(j == JT - 1))

                        def a_mms2(j):
                            for ct in range(CT):
                                nc.tensor.matmul(
                                    aps[ct][:, :],
                                    r(vt_sb[:, j, ct * P:(ct + 1) * P]),
                                    r(pts[j][:, :]),
                                    start=(j == 0), stop=(j == JT - 1))
                            a_mms_rs(j)

                        if jt > 2:
                            a_mms2(jt - 3)
                    a_mms2(JT - 3)
                    a_mms2(JT - 2)
                    a_mms2(JT - 1)
                    # Tail chain (rowsum combine -> recip -> broadcast) is
                    # the critical path at the end: emit it ahead of the as
                    # copies so it wins the DVE/PE queue slots.
                    if RS_PACK:
                        # rowsum = sum of the 4 packed partial rows: one wide
                        # copy of the zero-padded bank, one ones-contraction
                        rsc = astiles.tile([P, 512], F32, tag="rsc")
                        nc.vector.tensor_copy(rsc[:, :].bitcast(F32R),
                                              rsps[:, :])
                        nc.tensor.matmul(rsps[0:1, :], r(ones_sb[:, :]),
                                         r(rsc[:, :]),
                                         start=True, stop=True,
                                         skip_group_check=True)
                    recip = stats.tile([1, 512], F32, tag="recip")
                    nc.vector.reciprocal(out=recip[:, :],
                                         in_=rsps[0:1, :])
                    rb_sb = astiles.tile([P, 512], F32, tag="rbs")
                    nc.gpsimd.partition_broadcast(rb_sb[:, :], recip[:, :])
                    # Normalization by 1/rowsum is applied AFTER the projection
                    # (it commutes with the channel contraction), so the proj
                    # matmuls start as soon as A stops.
                    as_sb = astiles.tile([P, CT, 512], F32, tag="as")
                    for ct in range(CT):
                        nc.vector.tensor_copy(as_sb[:, ct, :].bitcast(F32R),
                                              aps[ct][:, :])
                    # projection; then out = proj*rb + (fbias + residual)
                    pps0 = psS.tile([P, 512], F32, tag="sps")
                    pps1 = psS.tile([P, 512], F32, tag="sps")
                    pps = (pps0, pps1)
                    for ct in range(CT):
                        for o in range(CT):
                            nc.tensor.matmul(
                                pps[o][:, :],
                                wp_sb[:, ct, o * P:(o + 1) * P],
                                r(as_sb[:, ct, :]),
                                start=(ct == 0), stop=(ct == CT - 1))
                    for o in range(CT):
                        nc.vector.tensor_tensor(
                            out=out_sb[:, o, isl], in0=pps[o][:, :],
                            in1=rb_sb[:, :], op=OP.mult)
                        nc.vector.scalar_tensor_tensor(
                            out=out_sb[:, o, isl], in0=out_sb[:, o, isl],
                            scalar=fb_sb[:, o:o + 1], in1=xq_sb[:, o, isl],
                            op0=OP.add, op1=OP.add)
                        nc.sync.dma_start(out=out_d[o, :, isl],
                                          in_=out_sb[:, o, isl])

            if dbg:
                for ct in range(CT):
                    nc.sync.dma_start(out=dbg_h[ct], in_=h_sb[:, ct, :])
                    nc.sync.dma_start(out=dbg_k[ct], in_=k_sb[:, ct, :])
                    nc.sync.dma_start(out=dbg_q[ct], in_=q_sb[:, ct, :])
                for jt in range(JT):
                    nc.sync.dma_start(out=dbg_vt[jt], in_=vt_sb[:, jt, :])

    nc.compile()
    return nc


_PROGRAM = None


def _get_program():
    global _PROGRAM
    if _PROGRAM is None:
        _PROGRAM = build_program()
    return _PROGRAM


def _in_maps(x):
    raise NotImplementedError  # replaced below; kept for clarity


def make_in_maps(x, gn_scale, gn_bias, wq, bq, wk, bk, wv, bv, wp, bp):
    x2 = np.ascontiguousarray(np.asarray(x, np.float32).reshape(B, C, N))
    cidx = np.arange(C)
    G_full = (cidx[:, None] // GSIZE == np.arange(NGROUPS)[None, :]).astype(np.float32)
    # bn_stats already averages over the free dim, so combining the GSIZE
    # per-channel (mean, E[x^2]) rows into a group stat divides by GSIZE only.
    csm = np.zeros((C, NGROUPS + 5), np.float32)
    csm[:, :NGROUPS] = G_full / GSIZE
    csm[:, NGROUPS + 0] = np.asarray(bq, np.float32)
    csm[:, NGROUPS + 1] = np.asarray(bp, np.float32)
    csm[:, NGROUPS + 2] = np.asarray(gn_scale, np.float32)
    csm[:, NGROUPS + 3] = np.asarray(gn_bias, np.float32)
    csm[:, NGROUPS + 4] = -np.asarray(gn_scale, np.float32)
    csm = np.ascontiguousarray(csm.reshape(CT, P, NGROUPS + 5))
    GT = np.ascontiguousarray(G_full.T)  # [32, 256]

    def wT(wm):
        return np.ascontiguousarray(np.asarray(wm, np.float32).T.reshape(CT, P, C))

    def col(v):
        return np.ascontiguousarray(np.asarray(v, np.float32).reshape(CT, P, 1))

    def col2(v):
        a = np.zeros((C, 2), np.float32)
        a[:, 0] = np.asarray(v, np.float32)
        return np.ascontiguousarray(a.reshape(CT, P, 2))

    shared = {
        "wqT": wT(wq), "wkT": wT(wk), "wvT": wT(wv), "wpT": wT(wp),
        "bv": col2(bv), "csm": csm, "GT": GT,
    }
    in_maps = []
    for core in range(8):
        bi, ci = divmod(core, 4)
        xb = np.ascontiguousarray(x2[bi].reshape(CT, P, N))
        xq = np.ascontiguousarray(
            x2[bi][:, ci * NQ:(ci + 1) * NQ].reshape(CT, P, NQ))
        in_maps.append(dict(shared, xb=xb, xq=xq))
    return in_maps


def run(in_maps, **kwargs):
    nc = _get_program()
    return run_bass_kernel_spmd(nc, in_maps, core_ids=list(range(8)), **kwargs)


def kernel(x, gn_scale, gn_bias, wq, bq, wk, bk, wv, bv, wp, bp):
    in_maps = make_in_maps(x, gn_scale, gn_bias, wq, bq, wk, bk, wv, bv, wp, bp)
    res = run(in_maps)
    out = np.empty((B, C, N), np.float32)
    for core in range(8):
        bi, ci = divmod(core, 4)
        out[bi][:, ci * NQ:(ci + 1) * NQ] = (
            res.results[core]["out"].reshape(C, NQ))
    return out.reshape(B, C, T, H, W)


if __name__ == "__main__":
    rng = np.random.default_rng(0)
    x = rng.standard_normal((B, C, T, H, W), dtype=np.float32)
    args = dict(
        x=x,
        gn_scale=np.ones(C, np.float32), gn_bias=np.zeros(C, np.float32),
        wq=rng.standard_normal((C, C), dtype=np.float32) / 16,
        bq=rng.standard_normal(C, dtype=np.float32) * 0.01,
        wk=rng.standard_normal((C, C), dtype=np.float32) / 16,
        bk=rng.standard_normal(C, dtype=np.float32) * 0.01,
        wv=rng.standard_normal((C, C), dtype=np.float32) / 16,
        bv=rng.standard_normal(C, dtype=np.float32) * 0.01,
        wp=rng.standard_normal((C, C), dtype=np.float32) / 16,
        bp=rng.standard_normal(C, dtype=np.float32) * 0.01,
    )
    out = kernel(**args)
    print("kernel ran, out shape", out.shape, "mean", float(out.mean()))


# revision 56
# speedup vs baseline: 1.0777x; 1.0034x over previous
"""NonLocalBlock (GroupNorm + 4096-token self-attention + proj + residual) on 8 TRN2 cores.

Sharding: core = (batch b in {0,1}, query-chunk q in {0..3}); each core holds its
batch's full x (needed for GN stats and K/V over all tokens) and computes the
output for its 1024-token query chunk. No collectives needed.

Math notes (exact reductions of the reference):
  - bk drops out: adding a per-j-constant... no -- adding k-bias shifts every
    logit of row i by q_i . bk, constant in j -> softmax invariant.
  - bv folds into the projection bias: softmax rows sum to 1, so
    proj(A + bv) = proj(A) + wp @ bv.
  - Normalization by the softmax row-sum commutes with the V- and P-matmuls,
    so we divide once on the small [c, i] result instead of the [i, j] matrix.
"""

import sys

for _p in ("/opt/trn_rl_repo",):
    if _p not in sys.path:
        sys.path.insert(0, _p)

import numpy as np

import concourse.bacc as bacc
import concourse.bass as bass
import concourse.tile as tile
from concourse import mybir
from concourse.bass_utils import run_bass_kernel_spmd

F32 = mybir.dt.float32
F32R = mybir.dt.float32r
AF = mybir.ActivationFunctionType
OP = mybir.AluOpType

B, C, T, H, W = 2, 256, 4, 32, 32
N = T * H * W            # 4096 tokens
NQ = N // 4              # 1024 query tokens per core
P = 128                  # partitions
CT = C // P              # 2 channel tiles
JT = N // P              # 32 key tiles of 128
NB = N // 512            # 8 key blocks of 512
IC = NQ // 512           # 2 query sub-chunks of 512
NGROUPS = 32
GSIZE = C // NGROUPS     # 8 channels per group
EPS = 1e-6
SCALE = C ** (-0.5)      # 1/16
# Pack the M=1 rowsum matmuls 4-at-a-time into disjoint PE column groups
# (tile_position) so they run concurrently -- each costs N cycles otherwise.
RS_PACK = False


def r(ap):
    """View an fp32 AP as float32r for full-rate PE matmuls (moving dim >= 256)."""
    return ap.bitcast(F32R)


def build_program(dbg=False):
    nc = bacc.Bacc("TRN2", target_bir_lowering=False, debug=False, num_devices=8)

    # ---- DRAM parameters (per core) ----
    xb_d = nc.declare_dram_parameter("xb", [CT, P, N], F32, isOutput=False)
    xq_d = nc.declare_dram_parameter("xq", [CT, P, NQ], F32, isOutput=False)
    wqT_d = nc.declare_dram_parameter("wqT", [CT, P, C], F32R, isOutput=False)
    wkT_d = nc.declare_dram_parameter("wkT", [CT, P, C], F32R, isOutput=False)
    wvT_d = nc.declare_dram_parameter("wvT", [CT, P, C], F32R, isOutput=False)
    wpT_d = nc.declare_dram_parameter("wpT", [CT, P, C], F32R, isOutput=False)
    # Packed small constants, one DMA: cols [0:32]=G group-indicator/GSIZE,
    # 32=bq, 33=bp, 34=gn_scale, 35=gn_bias.
    csm_d = nc.declare_dram_parameter("csm", [CT, P, NGROUPS + 5], F32,
                                      isOutput=False)
    bv_d = nc.declare_dram_parameter("bv", [CT, P, 2], F32R, isOutput=False)
    # GT[g, c] = (c//GSIZE == g)  (broadcast group stat back to channels)
    GT_d = nc.declare_dram_parameter("GT", [NGROUPS, C], F32, isOutput=False)
    out_d = nc.declare_dram_parameter("out", [CT, P, NQ], F32, isOutput=True)
    if dbg:
        dbg_h = nc.declare_dram_parameter("dbg_h", [CT, P, N], F32, isOutput=True)
        dbg_k = nc.declare_dram_parameter("dbg_k", [CT, P, N], F32, isOutput=True)
        dbg_vt = nc.declare_dram_parameter("dbg_vt", [JT, P, C], F32, isOutput=True)
        dbg_q = nc.declare_dram_parameter("dbg_q", [CT, P, NQ], F32, isOutput=True)
        dbg_s = nc.declare_dram_parameter("dbg_s", [P, 512], F32, isOutput=True)

    with tile.TileContext(nc) as tc:
        with (
            nc.allow_low_precision(reason="float32r rounding for full-rate PE"),
            tc.tile_pool(name="consts", bufs=1) as consts,
            tc.tile_pool(name="data", bufs=1) as data,
            tc.tile_pool(name="stats", bufs=1) as stats,
            tc.tile_pool(name="ptiles", bufs=8) as ptiles,
            tc.tile_pool(name="astiles", bufs=2) as astiles,
        ):
            # ---- input DMAs, one queue, explicit order by first-use time.
            # The ~330GB/s DMA pipe is the head bottleneck: small consts + wk
            # first (they gate the first PE ops), then the 4MB xb stream that
            # gates GN stats, then tensors needed progressively later.
            csm_sb = consts.tile([P, CT, NGROUPS + 5], F32, tag="csm")
            nc.sync.dma_start(out=csm_sb[:, :, :],
                              in_=csm_d.rearrange("ct p k -> p ct k"))
            G_sb = csm_sb[:, :, 0:NGROUPS]
            bq_sb = csm_sb[:, :, NGROUPS + 0]
            bp_sb = csm_sb[:, :, NGROUPS + 1]
            gsc_sb = csm_sb[:, :, NGROUPS + 2]
            gbi_sb = csm_sb[:, :, NGROUPS + 3]
            ngsc_sb = csm_sb[:, :, NGROUPS + 4]
            GT_sb = consts.tile([NGROUPS, C], F32, tag="GT")
            nc.sync.dma_start(out=GT_sb[:, :], in_=GT_d[:])
            # xb right behind the tiny stat constants: bn_stats consume chunks
            # at DMA rate, so the stats pipeline drains right after the last
            # chunk; everything else arrives just-in-time behind it.
            xb_sb = data.tile([P, CT, N], F32, tag="xb")      # raw x (stage 1 only)
            xq_sb = data.tile([P, CT, NQ], F32, tag="xq")
            for nb in range(NB):
                nsl = slice(nb * 512, (nb + 1) * 512)
                for ct in range(CT):
                    nc.sync.dma_start(out=xb_sb[:, ct, nsl], in_=xb_d[ct, :, nsl])
            wq_sb = consts.tile([P, CT, C], F32R, tag="wq")
            wk_sb = consts.tile([P, CT, C], F32R, tag="wk")
            wv_sb = consts.tile([P, CT, C], F32R, tag="wv")
            wp_sb = consts.tile([P, CT, C], F32R, tag="wp")
            nc.sync.dma_start(out=wk_sb[:, :, :],
                              in_=wkT_d.rearrange("ct p o -> p ct o"))
            nc.sync.dma_start(out=xq_sb[:, :, :],
                              in_=xq_d.rearrange("ct p i -> p ct i"))
            nc.sync.dma_start(out=wv_sb[:, :, :],
                              in_=wvT_d.rearrange("ct p o -> p ct o"))
            nc.sync.dma_start(out=wq_sb[:, :, :],
                              in_=wqT_d.rearrange("ct p o -> p ct o"))
            bv_sb = consts.tile([P, CT, 2], F32R, tag="bv")
            nc.sync.dma_start(out=bv_sb[:, :, :],
                              in_=bv_d.rearrange("ct p k -> p ct k"))
            nc.sync.dma_start(out=wp_sb[:, :, :],
                              in_=wpT_d.rearrange("ct p o -> p ct o"))
            ones_f = consts.tile([P, 1], F32, tag="ones_f")
            nc.vector.memset(ones_f[:, :], 1.0)
            ones_sb = consts.tile([P, 1], F32, tag="ones")
            nc.vector.tensor_copy(ones_sb[:, :].bitcast(F32R), ones_f[:, :])
            epsg_sb = consts.tile([NGROUPS, 1], F32, tag="epsg")
            nc.vector.memset(epsg_sb[:, :], EPS)

            # ---- big SBUF tensors ----
            h_sb = data.tile([P, CT, N], F32, tag="h")        # GN output
            hq_sb = data.tile([P, CT, NQ], F32, tag="hq")
            k_sb = data.tile([P, CT, N], F32, tag="k")        # K[o, j]
            # vt reuses xb's slot (same tag/size): xb is dead once h is built
            vt_sb = data.tile([P, JT, C], F32, tag="xb")      # V^T[j, o]
            q_sb = data.tile([P, CT, NQ], F32, tag="q")       # Q[o, i]
            out_sb = data.tile([P, CT, NQ], F32, tag="out")

            # ================= Stage 1: GroupNorm =================
            with tc.tile_pool(name="ps1", bufs=2, space="PSUM") as ps1:
                # PE warmup: the HAM clock gate halves the PE clock until it
                # has been busy ~3.4us. The PE is otherwise idle during the
                # xb DMA head, so run throwaway fp32 matmuls on early-arrived
                # data to enter stage 2 at full clock.
                wps = ps1.tile([P, 512], F32, tag="warm")
                for wi in range(5):
                    nc.tensor.matmul(
                        wps[0:NGROUPS + 5, :], csm_sb[:, 0, :],
                        xb_sb[:, 0, 0:512], start=True, stop=True,
                        skip_group_check=True)
                # per-channel mean/var over the 4096 free positions
                bst = stats.tile([P, CT, NB, 6], F32, tag="bst")
                mv = stats.tile([P, CT, 2], F32, tag="mv")
                mst = stats.tile([P, CT, 2], F32, tag="mst")   # (mean_c, E[x^2]_c)
                # nb-major to match DMA chunk arrival order (DVE is in-order)
                for nb in range(NB):
                    for ct in range(CT):
                        nc.vector.bn_stats(
                            out=bst[:, ct, nb, :],
                            in_=xb_sb[:, ct, nb * 512:(nb + 1) * 512],
                        )
                for ct in range(CT):
                    nc.vector.bn_aggr(out=mv[:, ct, :], in_=bst[:, ct, :, :])
                    nc.vector.tensor_copy(mst[:, ct, 0:1], mv[:, ct, 0:1])
                    # E[x^2] = var + mean^2
                    nc.vector.tensor_tensor(
                        out=mst[:, ct, 1:2], in0=mv[:, ct, 0:1],
                        in1=mv[:, ct, 0:1], op=OP.mult)
                    nc.vector.tensor_tensor(
                        out=mst[:, ct, 1:2], in0=mst[:, ct, 1:2],
                        in1=mv[:, ct, 1:2], op=OP.add)
                # group-sum across partitions: [g, (mean, Ex2)]
                gps = ps1.tile([NGROUPS, 2], F32, tag="gps")
                for ct in range(CT):
                    nc.tensor.matmul(gps[:, :], G_sb[:, ct, :], mst[:, ct, :],
                                     start=(ct == 0), stop=(ct == CT - 1))
                gmv = stats.tile([NGROUPS, 2], F32, tag="gmv")
                nc.vector.tensor_copy(gmv[:, :], gps[:, :])
                gtmp = stats.tile([NGROUPS, 1], F32, tag="gtmp")
                gvec = stats.tile([NGROUPS, 2], F32, tag="gvec")  # (mean_g, rstd_g)
                nc.vector.tensor_copy(gvec[:, 0:1], gmv[:, 0:1])
                # -var = mean^2 - E[x^2]; sqrt(var+eps) via scale=-1
                nc.vector.scalar_tensor_tensor(
                    out=gtmp, in0=gmv[:, 0:1], scalar=gmv[:, 0:1],
                    in1=gmv[:, 1:2], op0=OP.mult, op1=OP.subtract)
                nc.scalar.activation(out=gtmp, in_=gtmp, func=AF.Sqrt,
                                     bias=epsg_sb[:, :], scale=-1.0)
                nc.vector.reciprocal(out=gvec[:, 1:2], in_=gtmp)  # rstd_g
                # broadcast back to channels: cb[c, (mean, rstd)]
                svec = stats.tile([P, CT], F32, tag="svec")
                nsvec = stats.tile([P, CT], F32, tag="nsvec")
                tvec = stats.tile([P, CT], F32, tag="tvec")
                for ct in range(CT):
                    cps = ps1.tile([P, 2], F32, tag="cps")
                    nc.tensor.matmul(cps[:, :], GT_sb[:, ct * P:(ct + 1) * P],
                                     gvec[:, :], start=True, stop=True)
                    cb = stats.tile([P, 2], F32, tag="cb")
                    nc.vector.tensor_copy(cb[:, :], cps[:, :])
                    # s_c = rstd_c * gn_scale_c ; t_c = gn_bias_c + mean_c*(-s_c)
                    nc.vector.tensor_tensor(out=svec[:, ct:ct + 1], in0=cb[:, 1:2],
                                            in1=gsc_sb[:, ct, None], op=OP.mult)
                    nc.vector.tensor_tensor(out=nsvec[:, ct:ct + 1], in0=cb[:, 1:2],
                                            in1=ngsc_sb[:, ct, None], op=OP.mult)
                    nc.vector.scalar_tensor_tensor(
                        out=tvec[:, ct:ct + 1], in0=cb[:, 0:1],
                        scalar=nsvec[:, ct:ct + 1], in1=gbi_sb[:, ct, None],
                        op0=OP.mult, op1=OP.add)
                # h = s_c * x + t_c  (in place over xb; also hq from xq)
                for nb in range(NB):
                    nsl = slice(nb * 512, (nb + 1) * 512)
                    for ct in range(CT):
                        nc.scalar.activation(out=h_sb[:, ct, nsl].bitcast(F32R),
                                             in_=xb_sb[:, ct, nsl],
                                             func=AF.Identity,
                                             bias=tvec[:, ct:ct + 1],
                                             scale=svec[:, ct:ct + 1])
                for ct in range(CT):
                    nc.vector.tensor_scalar(
                        out=hq_sb[:, ct, :].bitcast(F32R), in0=xq_sb[:, ct, :],
                        scalar1=svec[:, ct:ct + 1], scalar2=tvec[:, ct:ct + 1],
                        op0=OP.mult, op1=OP.add)
            # ================= Stage 2: K, V^T, Q, proj-bias =================
            fb_sb = stats.tile([P, CT], F32, tag="fb")  # wp @ bv + bp
            with (
                tc.tile_pool(name="ps2", bufs=2, space="PSUM") as ps2,
                tc.tile_pool(name="ps2k", bufs=3, space="PSUM") as ps2k,
            ):
                def q_mms(o, ib):
                    qps = ps2.tile([P, 512], F32, tag="qps")
                    for ct in range(CT):
                        nc.tensor.matmul(
                            qps[:, :],
                            wq_sb[:, ct, o * P:(o + 1) * P],
                            r(hq_sb[:, ct, ib * 512:(ib + 1) * 512]),
                            start=(ct == 0), stop=(ct == CT - 1))
                    nc.scalar.activation(
                        out=q_sb[:, o, ib * 512:(ib + 1) * 512].bitcast(F32R),
                        in_=qps[:, :], func=AF.Identity,
                        bias=bq_sb[:, o, None], scale=1.0)

                # nb-major: K, V^T, Q interleaved along h-chunk readiness
                for nb in range(NB):
                    for o in range(CT):
                        kps = ps2k.tile([P, 512], F32, tag="kps")
                        for ct in range(CT):
                            nc.tensor.matmul(
                                kps[:, :],
                                wk_sb[:, ct, o * P:(o + 1) * P],
                                r(h_sb[:, ct, nb * 512:(nb + 1) * 512]),
                                start=(ct == 0), stop=(ct == CT - 1))
                        nc.vector.tensor_copy(
                            k_sb[:, o, nb * 512:(nb + 1) * 512].bitcast(F32R),
                            kps[:, :])
                    if nb == 0:
                        for o in range(CT):
                            for ib in range(IC):
                                q_mms(o, ib)
                for o in range(CT):
                    fps = ps2.tile([P, 2], F32, tag="qps")
                    for ct in range(CT):
                        nc.tensor.matmul(fps[:, :],
                                         wp_sb[:, ct, o * P:(o + 1) * P],
                                         bv_sb[:, ct, :],
                                         start=(ct == 0), stop=(ct == CT - 1))
                    nc.vector.tensor_tensor(out=fb_sb[:, o:o + 1], in0=fps[:, 0:1],
                                            in1=bp_sb[:, o, None], op=OP.add)

            # ================= Stage 3: attention per 512-query chunk =========
            with (
                tc.tile_pool(name="psA", bufs=1, space="PSUM") as psA,
                tc.tile_pool(name="psS", bufs=3, space="PSUM") as psS,
                tc.tile_pool(name="psV", bufs=2, space="PSUM") as psV,
            ):
                def vt_mms(jt):
                    # V^T tile production, interleaved into the ic0 attention
                    # loop: fills PE stall slots and gives the PSUM->SBUF
                    # copies slack
                    vps = psV.tile([P, C], F32, tag="vps")
                    for ct in range(CT):
                        nc.tensor.matmul(
                            vps[:, :],
                            r(h_sb[:, ct, jt * P:(jt + 1) * P]),
                            wv_sb[:, ct, :],
                            start=(ct == 0), stop=(ct == CT - 1))
                    nc.vector.tensor_copy(vt_sb[:, jt, :].bitcast(F32R),
                                          vps[:, :])
                for ic in range(IC):
                    if ic == 0:
                        for jt in range(7):
                            vt_mms(jt)
                    isl = slice(ic * 512, (ic + 1) * 512)
                    a0ps = psA.tile([P, 512], F32, tag="A0")
                    a1ps = psA.tile([P, 512], F32, tag="A1")
                    rsps = psA.tile([P, 512] if RS_PACK else [1, 512], F32,
                                    tag="rs")
                    if RS_PACK:
                        # zero the bank so only the 4 accumulator rows carry
                        # data; lets the end-of-loop combine be one wide copy
                        nc.vector.memset(rsps[:, :], 0.0)
                    aps = (a0ps, a1ps)
                    # software pipeline: S/exp of tile jt overlaps A-matmuls of
                    # tile jt-3 (exp latency fully hidden)
                    pts = [None] * JT
                    for jt in range(JT):
                        sps = psS.tile([P, 512], F32, tag="sps")
                        for o in range(CT):
                            nc.tensor.matmul(
                                sps[:, :],
                                r(k_sb[:, o, jt * P:(jt + 1) * P]),
                                r(q_sb[:, o, isl]),
                                start=(o == 0), stop=(o == CT - 1))
                        if dbg and ic == 0 and jt == 0:
                            dbg_s_sb = data.tile([P, 512], F32, tag="dbgs")
                            nc.vector.tensor_copy(dbg_s_sb[:, :], sps[:, :])
                            nc.sync.dma_start(out=dbg_s[:], in_=dbg_s_sb[:, :])
                        pt = ptiles.tile([P, 512], F32, tag="pt")
                        nc.scalar.activation(out=pt[:, :].bitcast(F32R), in_=sps[:, :],
                                             func=AF.Exp, bias=0.0, scale=SCALE)
                        pts[jt] = pt
                        if ic == 0 and jt + 7 < JT:
                            vt_mms(jt + 7)

                        def a_mms_rs(j):
                            if RS_PACK:
                                if j % 4 == 3:
                                    # 4 back-to-back M=1 matmuls in distinct
                                    # column groups -> concurrent on the PE
                                    for k in range(4):
                                        jj = j - 3 + k
                                        nc.tensor.matmul(
                                            rsps[32 * k:32 * k + 1, :],
                                            r(ones_sb[:, :]),
                                            r(pts[jj][:, :]),
                                            start=(jj < 4), stop=(jj >= JT - 4),
                                            tile_position=(0, 32 * k),
                                            skip_group_check=True)
                            else:
                                nc.tensor.matmul(rsps[:, :], r(ones_sb[:, :]),
                                                 r(pts[j][:, :]),
                                                 start=(j == 0), stop=(j == JT - 1))

                        def a_mms2(j):
                            for ct in range(CT):
                                nc.tensor.matmul(
                                    aps[ct][:, :],
                                    r(vt_sb[:, j, ct * P:(ct + 1) * P]),
                                    r(pts[j][:, :]),
                                    start=(j == 0), stop=(j == JT - 1))
                            a_mms_rs(j)

                        if jt > 2:
                            a_mms2(jt - 3)
                    a_mms2(JT - 3)
                    a_mms2(JT - 2)
                    a_mms2(JT - 1)
                    # Tail chain (rowsum combine -> recip -> broadcast) is
                    # the critical path at the end: emit it ahead of the as
                    # copies so it wins the DVE/PE queue slots.
                    if RS_PACK:
                        # rowsum = sum of the 4 packed partial rows: one wide
                        # copy of the zero-padded bank, one ones-contraction
                        rsc = astiles.tile([P, 512], F32, tag="rsc")
                        nc.vector.tensor_copy(rsc[:, :].bitcast(F32R),
                                              rsps[:, :])
                        nc.tensor.matmul(rsps[0:1, :], r(ones_sb[:, :]),
                                         r(rsc[:, :]),
                                         start=True, stop=True,
                                         skip_group_check=True)
                    recip = stats.tile([1, 512], F32, tag="recip")
                    nc.vector.reciprocal(out=recip[:, :],
                                         in_=rsps[0:1, :])
                    rb_sb = astiles.tile([P, 512], F32, tag="rbs")
                    nc.gpsimd.partition_broadcast(rb_sb[:, :], recip[:, :])
                    # Normalization by 1/rowsum is applied AFTER the projection
                    # (it commutes with the channel contraction), so the proj
                    # matmuls start as soon as A stops.
                    as_sb = astiles.tile([P, CT, 512], F32, tag="as")
                    for ct in range(CT):
                        nc.vector.tensor_copy(as_sb[:, ct, :].bitcast(F32R),
                                              aps[ct][:, :])
                    # projection; then out = proj*rb + (fbias + residual)
                    pps0 = psS.tile([P, 512], F32, tag="sps")
                    pps1 = psS.tile([P, 512], F32, tag="sps")
                    pps = (pps0, pps1)
                    for ct in range(CT):
                        for o in range(CT):
                            nc.tensor.matmul(
                                pps[o][:, :],
                                wp_sb[:, ct, o * P:(o + 1) * P],
                                r(as_sb[:, ct, :]),
                                start=(ct == 0), stop=(ct == CT - 1))
                    for o in range(CT):
                        nc.vector.tensor_tensor(
                            out=out_sb[:, o, isl], in0=pps[o][:, :],
                            in1=rb_sb[:, :], op=OP.mult)
                        nc.vector.scalar_tensor_tensor(
                            out=out_sb[:, o, isl], in0=out_sb[:, o, isl],
                            scalar=fb_sb[:, o:o + 1], in1=xq_sb[:, o, isl],
                            op0=OP.add, op1=OP.add)
                        nc.sync.dma_start(out=out_d[o, :, isl],
                                          in_=out_sb[:, o, isl])

            if dbg:
                for ct in range(CT):
                    nc.sync.dma_start(out=dbg_h[ct], in_=h_sb[:, ct, :])
                    nc.sync.dma_start(out=dbg_k[ct], in_=k_sb[:, ct, :])
                    nc.sync.dma_start(out=dbg_q[ct], in_=q_sb[:, ct, :])
                for jt in range(JT):
                    nc.sync.dma_start(out=dbg_vt[jt], in_=vt_sb[:, jt, :])

    nc.compile()
    return nc


_PROGRAM = None


def _get_program():
    global _PROGRAM
    if _PROGRAM is None:
        _PROGRAM = build_program()
    return _PROGRAM


def _in_maps(x):
    raise NotImplementedError  # replaced below; kept for clarity


def make_in_maps(x, gn_scale, gn_bias, wq, bq, wk, bk, wv, bv, wp, bp):
    x2 = np.ascontiguousarray(np.asarray(x, np.float32).reshape(B, C, N))
    cidx = np.arange(C)
    G_full = (cidx[:, None] // GSIZE == np.arange(NGROUPS)[None, :]).astype(np.float32)
    # bn_stats already averages over the free dim, so combining the GSIZE
    # per-channel (mean, E[x^2]) rows into a group stat divides by GSIZE only.
    csm = np.zeros((C, NGROUPS + 5), np.float32)
    csm[:, :NGROUPS] = G_full / GSIZE
    csm[:, NGROUPS + 0] = np.asarray(bq, np.float32)
    csm[:, NGROUPS + 1] = np.asarray(bp, np.float32)
    csm[:, NGROUPS + 2] = np.asarray(gn_scale, np.float32)
    csm[:, NGROUPS + 3] = np.asarray(gn_bias, np.float32)
    csm[:, NGROUPS + 4] = -np.asarray(gn_scale, np.float32)
    csm = np.ascontiguousarray(csm.reshape(CT, P, NGROUPS + 5))
    GT = np.ascontiguousarray(G_full.T)  # [32, 256]

    def wT(wm):
        return np.ascontiguousarray(np.asarray(wm, np.float32).T.reshape(CT, P, C))

    def col(v):
        return np.ascontiguousarray(np.asarray(v, np.float32).reshape(CT, P, 1))

    def col2(v):
        a = np.zeros((C, 2), np.float32)
        a[:, 0] = np.asarray(v, np.float32)
        return np.ascontiguousarray(a.reshape(CT, P, 2))

    shared = {
        "wqT": wT(wq), "wkT": wT(wk), "wvT": wT(wv), "wpT": wT(wp),
        "bv": col2(bv), "csm": csm, "GT": GT,
    }
    in_maps = []
    for core in range(8):
        bi, ci = divmod(core, 4)
        xb = np.ascontiguousarray(x2[bi].reshape(CT, P, N))
        xq = np.ascontiguousarray(
            x2[bi][:, ci * NQ:(ci + 1) * NQ].reshape(CT, P, NQ))
        in_maps.append(dict(shared, xb=xb, xq=xq))
    return in_maps


def run(in_maps, **kwargs):
    nc = _get_program()
    return run_bass_kernel_spmd(nc, in_maps, core_ids=list(range(8)), **kwargs)


def kernel(x, gn_scale, gn_bias, wq, bq, wk, bk, wv, bv, wp, bp):
    in_maps = make_in_maps(x, gn_scale, gn_bias, wq, bq, wk, bk, wv, bv, wp, bp)
    res = run(in_maps)
    out = np.empty((B, C, N), np.float32)
    for core in range(8):
        bi, ci = divmod(core, 4)
        out[bi][:, ci * NQ:(ci + 1) * NQ] = (
            res.results[core]["out"].reshape(C, NQ))
    return out.reshape(B, C, T, H, W)


if __name__ == "__main__":
    rng = np.random.default_rng(0)
    x = rng.standard_normal((B, C, T, H, W), dtype=np.float32)
    args = dict(
        x=x,
        gn_scale=np.ones(C, np.float32), gn_bias=np.zeros(C, np.float32),
        wq=rng.standard_normal((C, C), dtype=np.float32) / 16,
        bq=rng.standard_normal(C, dtype=np.float32) * 0.01,
        wk=rng.standard_normal((C, C), dtype=np.float32) / 16,
        bk=rng.standard_normal(C, dtype=np.float32) * 0.01,
        wv=rng.standard_normal((C, C), dtype=np.float32) / 16,
        bv=rng.standard_normal(C, dtype=np.float32) * 0.01,
        wp=rng.standard_normal((C, C), dtype=np.float32) / 16,
        bp=rng.standard_normal(C, dtype=np.float32) * 0.01,
    )
    out = kernel(**args)
    print("kernel ran, out shape", out.shape, "mean", float(out.mean()))
